# revision 10
# baseline (speedup 1.0000x reference)
"""Causal GQA attention (B=2, T=2048, C=2048, H=16, HKV=4, D=128, RoPE)
on 8 Trainium2 NeuronCores — v2.

Sharding: core c = (batch b = c//4, kv-group g = c%4): 4 q heads + 1 kv
head per core; row-parallel output projection, host sums 4 partials.

Design notes:
  - Projections run as fp8e4 hi/lo DoubleRow matmuls (3 compensation
    terms over k-tile pairs). x and w ship pre-split/pre-scaled; dequant
    folds into the RoPE tables (q/k), the v-copy scale, and the softmax
    scale applied at exp time.
  - bf16 storage elsewhere; fp8 only where noise is attenuated
    (off-diagonal softmax weights + hi/lo v).
  - Attention is block-causal at 128 granularity via suffix-truncated
    moving operands on the diagonal; off-diagonal s-tile pairs exp
    straight to fp8 and feed DoubleRow PV (v hi/lo) and DoubleRow
    ones-matmul denominators.
  - Output projection (bf16) is interleaved into phase B's instruction
    stream to fill the tensor engine during Act-bound stretches.
  - DMA pacing: packed projection-weight stream, wo deferred to phase
    B, trig tables per-chunk bf16 with pool-rotation gating, x
    triple-buffered.
  - RoPE: a partition-pre-swapped bf16 staging copy (Act; PSUM input
    permits the partition offset, SBUF x SBUF would not) lets the sin
    multiply run as one aligned 2-byte-mode DVE op.
"""

import os
from contextlib import ExitStack

import numpy as np

import concourse.bass as bass
import concourse.tile as tile
from concourse import bacc, mybir
from concourse.bass_utils import run_bass_kernel_spmd
from concourse.masks import make_identity

B, T, C = 2, 2048, 2048
H, HKV, D = 16, 4, 128
GROUP = H // HKV
THETA = 1000000.0
SCALE = D ** -0.5

P = 128
TCH = 512
NJT = T // TCH             # 4
NK = C // P                # 16 k-tiles
NKP = NK // 2              # 8 DoubleRow k-tile pairs
NH = GROUP                 # 4 local q heads
NST = T // P               # 16 s-tiles
N_CORES = 8

SX = 16.0                  # x fp8 scale
SW = 1024.0                # w fp8 scale (q/k/v)
SV = 2.0 ** -10            # v psum -> sbuf scale
CDEN = 0.25                # ones constant: ot = 64*O_true (fp8 range)
SWO = 1024.0               # wo fp8 scale
YDQ = 1.0 / (64.0 * SWO)   # host-side dequant of the y partials

F32 = mybir.dt.float32
BF16 = mybir.dt.bfloat16
F8 = mybir.dt.float8e4
DR = mybir.MatmulPerfMode.DoubleRow


def build_program(phases="ABC", variant=""):
    nc = bacc.Bacc("TRN2", target_bir_lowering=False, debug=False)

    xh_d = nc.dram_tensor("xh", [C, T], F8, kind="ExternalInput").ap()
    xl_d = nc.dram_tensor("xl", [C, T], F8, kind="ExternalInput").ap()
    WPK = NH * D + 2 * D       # packed row: wq 512 | wk 128 | wv 128
    wAh_d = nc.dram_tensor("wAh", [P, NKP * 2 * WPK], F8, kind="ExternalInput").ap()
    wAl_d = nc.dram_tensor("wAl", [P, NKP * 2 * WPK], F8, kind="ExternalInput").ap()
    woh_d = nc.dram_tensor("woh", [P, 2 * NJT * 2 * TCH], F8, kind="ExternalInput").ap()
    wol_d = nc.dram_tensor("wol", [P, 2 * NJT * 2 * TCH], F8, kind="ExternalInput").ap()
    cos_d = nc.dram_tensor("cosT", [P, T], BF16, kind="ExternalInput").ap()
    sin_d = nc.dram_tensor("sinT", [P, T], BF16, kind="ExternalInput").ap()
    tri_d = nc.dram_tensor("tri", [P, P], BF16, kind="ExternalInput").ap()
    on8_d = nc.dram_tensor("on8", [P, 256], F8, kind="ExternalInput").ap()
    onb_d = nc.dram_tensor("onb", [P, P], BF16, kind="ExternalInput").ap()
    y_d = nc.dram_tensor("y", [T, C], BF16, kind="ExternalOutput").ap()

    with tile.TileContext(nc) as tc, ExitStack() as ctx:
        wpool = ctx.enter_context(tc.tile_pool(name="weights", bufs=1))
        tpool = ctx.enter_context(tc.tile_pool(name="tables", bufs=1))
        trigp = ctx.enter_context(tc.tile_pool(name="trig", bufs=2))
        state = ctx.enter_context(tc.tile_pool(name="state", bufs=1))

        wAh_sb = wpool.tile([P, NKP, 2, WPK], F8, tag="wAh")
        wAl_sb = wpool.tile([P, NKP, 2, WPK], F8, tag="wAl")
        for h4 in range(4):
            nc.scalar.dma_start(
                wAh_sb[:, 2 * h4:2 * h4 + 2, :, :],
                wAh_d[:, 2 * h4 * 2 * WPK:(2 * h4 + 2) * 2 * WPK].rearrange(
                    "p (kp two o) -> p kp two o", kp=2, two=2))
        for h4 in range(4):
            nc.scalar.dma_start(
                wAl_sb[:, 2 * h4:2 * h4 + 2, :, :],
                wAl_d[:, 2 * h4 * 2 * WPK:(2 * h4 + 2) * 2 * WPK].rearrange(
                    "p (kp two o) -> p kp two o", kp=2, two=2))

        tri_sb = tpool.tile([P, P], BF16, tag="tri")
        nc.gpsimd.dma_start(tri_sb[:], tri_d[:])
        on8_sb = tpool.tile([P, 2, P], F8, tag="on8")
        nc.gpsimd.dma_start(on8_sb[:], on8_d.rearrange("p (two f) -> p two f", two=2))
        onb_sb = tpool.tile([P, P], BF16, tag="onb")
        nc.gpsimd.dma_start(onb_sb[:], onb_d[:])
        identb = tpool.tile([P, P], BF16, tag="ident")
        make_identity(nc, identb[:])
        # (variant parsed below, before use)

        # PE warmup: keep the tensor engine continuously busy from t~0 so the
        # p-state ramp completes before the first real (DMA-gated) matmuls.
        warmp = ctx.enter_context(tc.tile_pool(name="warm", bufs=1))
        warm_sb = warmp.tile([P, P], BF16, tag="w")
        nc.vector.memset(warm_sb[:], 0.0)
        nwarm = 0
        for v in variant.split("+"):
            if v.startswith("warm"):
                nwarm = int(v[4:])
        with tc.tile_pool(name="psW", bufs=1, space="PSUM") as psW:
            wp = psW.tile([P, P], F32, tag="wp")
            for i in range(nwarm):
                nc.tensor.matmul(wp[:], identb[:], warm_sb[:],
                                 start=(i == 0), stop=(i == nwarm - 1))

        qrot = state.tile([P, NH, T], BF16, tag="qrot")
        krot = state.tile([P, T], BF16, tag="krot")
        v_sb = state.tile([P, NST, D], BF16, tag="v")
        v8h = state.tile([P, NST // 2, 2, D], F8, tag="v8h")
        v8l = state.tile([P, NST // 2, 2, D], F8, tag="v8l")
        ot8h = state.tile([P, 2, NST, 2, D], F8, tag="ot8h")
        ot8l = state.tile([P, 2, NST, 2, D], F8, tag="ot8l")

        njt_lim = NJT
        for v in variant.split("+"):
            if v.startswith("njt"):
                njt_lim = int(v[3:])

        # ---------------- Phase A: projections + RoPE -----------------
        with ExitStack() as actx:
          if "A" in phases:
            xpool = actx.enter_context(tc.tile_pool(name="xsub", bufs=3))
            ropep = actx.enter_context(tc.tile_pool(name="rope", bufs=3))
            vtp = actx.enter_context(tc.tile_pool(name="vt", bufs=2))
            psA = actx.enter_context(tc.tile_pool(name="psA", bufs=1, space="PSUM"))
            psT = actx.enter_context(tc.tile_pool(name="psT", bufs=2, space="PSUM"))

            def rope(acc_ps, out_ap, cos_t, sin_t):
                # partition-pre-swapped bf16 stage (Act, PSUM input permits the
                # offset) so both DVE multiplies run aligned in 2-byte mode
                absw = ropep.tile([P, TCH], BF16, tag="ab")
                nc.scalar.copy(absw[0:64, :], acc_ps[64:128, :])
                nc.scalar.copy(absw[64:128, :], acc_ps[0:64, :])
                m1 = ropep.tile([P, TCH], BF16, tag="m1")
                m2 = ropep.tile([P, TCH], BF16, tag="m2")
                nc.vector.tensor_tensor(
                    m1[:], acc_ps[:], cos_t[:], mybir.AluOpType.mult)
                nc.vector.tensor_tensor(
                    m2[:], absw[:], sin_t[:], mybir.AluOpType.mult)
                nc.vector.tensor_tensor(
                    out_ap, m1[:], m2[:], mybir.AluOpType.add)

            def w_slice(term, o, m):
                sb = wAh_sb if term != 2 else wAl_sb
                if o < NH:
                    return sb[:, m, :, o * D:(o + 1) * D]
                if o == NH:
                    return sb[:, m, :, NH * D:NH * D + D]
                return sb[:, m, :, NH * D + D:NH * D + 2 * D]

            def finish(o, acc, jt, cos_t, sin_t):
                """Returns a deferred-emission thunk (or None)."""
                if "noropeA" in variant:
                    return None
                if o < NH:
                    rope(acc, qrot[:, o, jt * TCH:(jt + 1) * TCH], cos_t, sin_t)
                    return None
                if o == NH:
                    rope(acc, krot[:, jt * TCH:(jt + 1) * TCH], cos_t, sin_t)
                    return None
                vt = vtp.tile([P, TCH], BF16, tag="vt")
                nc.scalar.activation(
                    vt[:], acc[:], mybir.ActivationFunctionType.Copy, scale=SV)

                def transposes():
                    for i in range(TCH // P):
                        s_idx = jt * (TCH // P) + i
                        pst = psT.tile([P, P], BF16, tag="pst")
                        nc.tensor.transpose(pst[:], vt[:, i * P:(i + 1) * P],
                                            identb[:])
                        nc.scalar.copy(v_sb[:, s_idx, :], pst[:])
                        nc.scalar.copy(v8h[:, s_idx // 2, s_idx % 2, :], pst[:])
                        nc.vector.tensor_tensor(
                            v8l[:, s_idx // 2, s_idx % 2, :], pst[:],
                            v8h[:, s_idx // 2, s_idx % 2, :],
                            mybir.AluOpType.subtract)
                return transposes

            nacc = 0
            deferred = None
            for jt in range(njt_lim):
                cos_t = trigp.tile([P, TCH], BF16, tag="cos")
                sin_t = trigp.tile([P, TCH], BF16, tag="sin")
                ch = slice(jt * TCH, (jt + 1) * TCH)
                nc.gpsimd.dma_start(cos_t[:], cos_d[:, ch])
                nc.gpsimd.dma_start(sin_t[:], sin_d[:, ch])

                xhs, xls = [], []
                for m in range(NKP):
                    xt = xpool.tile([P, 2, TCH], F8, tag=f"xh{m}")
                    nc.sync.dma_start(
                        xt[:],
                        xh_d[2 * m * P:(2 * m + 2) * P, ch].rearrange(
                            "(two p) t -> p two t", p=P))
                    xhs.append(xt)
                for m in range(NKP):
                    xt = xpool.tile([P, 2, TCH], F8, tag=f"xl{m}")
                    nc.sync.dma_start(
                        xt[:],
                        xl_d[2 * m * P:(2 * m + 2) * P, ch].rearrange(
                            "(two p) t -> p two t", p=P))
                    xls.append(xt)

                if jt == 0:
                    accs = [psA.tile([P, TCH], F32, tag=f"acc{o}",
                                     name=f"acc{o}")
                            for o in range(6)]
                    for ti, (xs, term) in enumerate(((xhs, 0), (xls, 1),
                                                     (xhs, 2))):
                        for m in range(NKP):
                            for o in (4, 5, 0, 1, 2, 3):
                                nc.tensor.matmul(
                                    accs[o][:], w_slice(term, o, m),
                                    xs[m][:],
                                    start=(ti == 0 and m == 0),
                                    stop=(ti == 2 and m == NKP - 1),
                                    perf_mode=DR)
                    for o in (5, 4, 0, 1, 2, 3):
                        d = finish(o, accs[o], jt, cos_t, sin_t)
                        if d is not None:
                            deferred = d
                    nacc = 5
                else:
                    for o in (4, 5, 0, 1, 2, 3):
                        acc = psA.tile([P, TCH], F32, tag=f"acc{nacc % 6}",
                                       name=f"accr{nacc % 6}")
                        nacc += 1
                        first = True
                        for xs, term in (((xhs, 0), (xls, 1), (xhs, 2))):
                            for m in range(NKP):
                                nc.tensor.matmul(
                                    acc[:], w_slice(term, o, m), xs[m][:],
                                    start=first,
                                    stop=(term == 2 and m == NKP - 1),
                                    perf_mode=DR)
                                first = False
                        if deferred is not None:
                            deferred()
                            deferred = None
                        d = finish(o, acc, jt, cos_t, sin_t)
                        if d is not None:
                            deferred = d
                if jt == njt_lim - 1 and deferred is not None:
                    deferred()
                    deferred = None

        # ---------------- Phase B + C interleaved ----------------------
        with ExitStack() as bctx:
          if "B" in phases:
            e8p = bctx.enter_context(tc.tile_pool(name="e8", bufs=6))
            edp = bctx.enter_context(tc.tile_pool(name="ed", bufs=3))
            rcp = bctx.enter_context(tc.tile_pool(name="rc", bufs=3))
            wopool = bctx.enter_context(tc.tile_pool(name="wo", bufs=1))
            ypool = bctx.enter_context(tc.tile_pool(name="ysb", bufs=6))
            psS = bctx.enter_context(tc.tile_pool(name="psS", bufs=2, space="PSUM"))
            psO = bctx.enter_context(tc.tile_pool(name="psO", bufs=1, space="PSUM"))
            psD = bctx.enter_context(tc.tile_pool(name="psD", bufs=1, space="PSUM"))
            psC = bctx.enter_context(tc.tile_pool(name="psC", bufs=2, space="PSUM"))

            woh_sb = wopool.tile([P, 2, NJT, 2, TCH], F8, tag="woh")
            wol_sb = wopool.tile([P, 2, NJT, 2, TCH], F8, tag="wol")
            nc.sync.dma_start(
                woh_sb[:], woh_d.rearrange("p (i jc two t) -> p i jc two t",
                                           i=2, jc=NJT, two=2))
            nc.sync.dma_start(
                wol_sb[:], wol_d.rearrange("p (i jc two t) -> p i jc two t",
                                           i=2, jc=NJT, two=2))

            def c_group(tt, jc):
                yp = psC.tile([P, TCH], F32, tag="y")
                first = True
                for i in range(2):
                    for lhs, rhs in ((ot8h, woh_sb), (ot8l, woh_sb),
                                     (ot8h, wol_sb)):
                        nc.tensor.matmul(
                            yp[:], lhs[:, i, tt, :, :], rhs[:, i, jc, :, :],
                            start=first, stop=(i == 1 and rhs is wol_sb),
                            perf_mode=DR, skip_group_check=True)
                        first = False
                ys = ypool.tile([P, TCH], BF16, tag="ys")
                nc.vector.tensor_copy(ys[:], yp[:])
                nc.sync.dma_start(
                    y_d[tt * P:(tt + 1) * P, jc * TCH:(jc + 1) * TCH],
                    ys[:])

            do_c = "C" in phases

            nbjt = NJT
            for v in variant.split("+"):
                if v.startswith("nbjt"):
                    nbjt = int(v[4:])
            cpend = []

            def emit_c(n=1):
                for _ in range(n):
                    if cpend:
                        c_group(*cpend.pop(0))

            for jt in range(nbjt):
                ch = slice(jt * TCH, (jt + 1) * TCH)
                if do_c and jt >= 1:
                    cpend = [(4 * (jt - 1) + tt4, jc)
                             for tt4 in range(4) for jc in range(NJT)]
                for h in range(NH):
                    qch = qrot[:, h, ch]
                    ot_ps = psO.tile([P, TCH], F32, tag="ot")
                    dn_ps = psD.tile([P, TCH], F32, tag="dn")
                    npair = 2 * jt

                    def sc_pair(m):
                        sps = psS.tile([P, 2 * TCH], F32, tag="su")
                        for i in range(2):
                            js = 2 * m + i
                            nc.tensor.matmul(
                                sps[:, i * TCH:(i + 1) * TCH],
                                krot[:, js * P:(js + 1) * P], qch,
                                start=True, stop=True)
                        e8 = e8p.tile([P, 2, TCH], F8, tag="e8")
                        nc.scalar.activation(
                            e8[:], sps[:], mybir.ActivationFunctionType.Exp,
                            scale=SCALE)
                        return e8

                    def pv_pair(m, e8, start):
                        nc.tensor.matmul(
                            ot_ps[:], v8h[:, m, :, :], e8[:],
                            start=start, stop=False, perf_mode=DR,
                            skip_group_check=True)
                        nc.tensor.matmul(
                            ot_ps[:], v8l[:, m, :, :], e8[:],
                            start=False, stop=False, perf_mode=DR,
                            skip_group_check=True)
                        nc.tensor.matmul(
                            dn_ps[:], on8_sb[:], e8[:],
                            start=start, stop=False, perf_mode=DR,
                            skip_group_check=True)

                    pend = []
                    for m in range(npair):
                        e8 = sc_pair(m)
                        if len(pend) >= 4:
                            mm, ee = pend.pop(0)
                            pv_pair(mm, ee, mm == 0)
                        pend.append((m, e8))

                    dsup = []
                    for half in range(2):
                        sps = psS.tile([P, 2 * TCH], F32, tag="su")
                        ed = edp.tile([P, 2 * TCH], BF16, tag="ed")
                        widths = []
                        off = 0
                        for rr in range(2):
                            r = 2 * half + rr
                            w = TCH - r * P
                            js = 4 * jt + r
                            nc.tensor.matmul(
                                sps[:, off:off + w],
                                krot[:, js * P:(js + 1) * P],
                                qch[:, r * P:],
                                start=True, stop=True)
                            widths.append((r, off, w))
                            off += w
                        nc.scalar.activation(
                            ed[:, 0:off], sps[:, 0:off],
                            mybir.ActivationFunctionType.Exp, scale=SCALE)
                        for r, off_, w in widths:
                            nc.vector.tensor_tensor(
                                ed[:, off_:off_ + P], ed[:, off_:off_ + P],
                                tri_sb[:], mybir.AluOpType.mult)
                        dsup.append((ed, widths))
                        while pend:
                            mm, ee = pend.pop(0)
                            pv_pair(mm, ee, mm == 0)

                    for ed, widths in dsup:
                        for r, off, w in widths:
                            js = 4 * jt + r
                            nc.tensor.matmul(
                                ot_ps[:, r * P:], v_sb[:, js, :],
                                ed[:, off:off + w],
                                start=(jt == 0 and r == 0), stop=(r == 3),
                                skip_group_check=True)
                            nc.tensor.matmul(
                                dn_ps[:, r * P:], onb_sb[:],
                                ed[:, off:off + w],
                                start=(jt == 0 and r == 0), stop=(r == 3),
                                skip_group_check=True)

                    if not (jt == NJT - 1 and h == NH - 1):
                        emit_c(4)
                    rb = rcp.tile([P, TCH], F32, tag="rb")
                    nc.vector.reciprocal(rb[:], dn_ps[:])
                    tmp = rcp.tile([P, 4, P], F32, tag="tmp")
                    nc.vector.tensor_tensor(
                        tmp[:].rearrange("p a b -> p (a b)"), ot_ps[:], rb[:],
                        mybir.AluOpType.mult)
                    oh = ot8h[:, h // 2, 4 * jt:4 * jt + 4, h % 2, :]
                    ol = ot8l[:, h // 2, 4 * jt:4 * jt + 4, h % 2, :]
                    nc.vector.tensor_copy(oh, tmp[:])
                    nc.vector.tensor_tensor(
                        ol, tmp[:], oh, mybir.AluOpType.subtract)
                    if jt == NJT - 1 and h == NH - 1:
                        emit_c(4)


            if do_c:
                for tt4 in range(4):
                    for jc in range(NJT):
                        c_group(12 + tt4, jc)

    nc.compile()
    return nc


def host_prep(x, wq, wk, wv, wo):
    import ml_dtypes
    F8np = ml_dtypes.float8_e4m3
    BFnp = ml_dtypes.bfloat16

    x = np.asarray(x, dtype=np.float32)
    wq = np.asarray(wq, dtype=np.float32)
    wk = np.asarray(wk, dtype=np.float32)
    wv = np.asarray(wv, dtype=np.float32)
    wo = np.asarray(wo, dtype=np.float32)

    perm = np.concatenate([np.arange(0, D, 2), np.arange(1, D, 2)])

    inv_freq = (1.0 / THETA ** (np.arange(0, D, 2, dtype=np.float32) / D)).astype(np.float32)
    pos = np.arange(T, dtype=np.float32)
    freqs = pos[:, None] * inv_freq[None, :]
    cos_t = np.cos(freqs).astype(np.float32).T
    sin_t = np.sin(freqs).astype(np.float32).T
    dq = np.float32(1.0 / (SX * SW))
    cosT = np.concatenate([cos_t, cos_t], axis=0) * dq
    sinT = np.concatenate([-sin_t, sin_t], axis=0) * dq

    tri = (np.arange(P)[None, :] >= np.arange(P)[:, None]).astype(BFnp)

    def hilo(a):
        h = a.astype(F8np)
        l = (a - h.astype(np.float32)).astype(F8np)
        return h, l

    xs = [np.ascontiguousarray(x[b].T) * SX for b in range(B)]
    xhl = [hilo(a) for a in xs]

    in_maps = []
    for c in range(N_CORES):
        b, g = divmod(c, GROUP)
        rows = []
        for hh in range(NH):
            h = g * GROUP + hh
            rows.append(wq[h * D + perm, :])
        wq_g = np.concatenate(rows, axis=0) * SW          # [512, C]
        wk_g = wk[g * D + perm, :] * SW
        wv_g = wv[g * D:(g + 1) * D, :] * SW
        wo_g = wo[:, g * NH * D:(g + 1) * NH * D]         # [C, 512]

        wqT = np.ascontiguousarray(wq_g.T)                # [C, 512]
        wkT = np.ascontiguousarray(wk_g.T)                # [C, 128]
        wvT = np.ascontiguousarray(wv_g.T)                # [C, 128]
        rows = np.arange(C).reshape(NKP, 2, P)            # [kp, two, p]
        pk = np.concatenate([wqT[rows], wkT[rows], wvT[rows]], axis=-1)
        pk = np.ascontiguousarray(np.transpose(pk, (2, 0, 1, 3)))  # [p,kp,two,col]
        wAh, wAl = hilo(pk.reshape(P, -1))
        # wo pair-contiguous DR layout: [d, hpair, jc, h-in-pair, tc]
        woT = np.ascontiguousarray(wo_g.T) * SWO          # [512, C]
        wo_a = woT.reshape(2, 2, D, NJT, TCH)             # [i, hh, d, jc, tc]
        wo_b = np.ascontiguousarray(np.transpose(wo_a, (2, 0, 3, 1, 4)))
        woh, wol = hilo(wo_b.reshape(P, -1))

        in_maps.append({
            "xh": xhl[b][0], "xl": xhl[b][1],
            "wAh": wAh, "wAl": wAl,
            "woh": woh, "wol": wol,
            "cosT": cosT.astype(BFnp),
            "sinT": sinT.astype(BFnp),
            "tri": tri,
            "on8": np.full((P, 256), CDEN, dtype=F8np),
            "onb": np.full((P, P), CDEN, dtype=BFnp),
        })
    return in_maps


_CACHE = {}


def _get_program(key="v2"):
    if key not in _CACHE:
        _CACHE[key] = build_program()
    return _CACHE[key]


def kernel(x, mask, wq, wk, wv, wo):
    nc = _get_program()
    in_maps = host_prep(x, wq, wk, wv, wo)
    res = run_bass_kernel_spmd(nc, in_maps, list(range(N_CORES))).results
    out = np.zeros((B, T, C), dtype=np.float32)
    for c in range(N_CORES):
        out[c // GROUP] += res[c]["y"].astype(np.float32) * YDQ
    return out


# revision 11
# speedup vs baseline: 1.0405x; 1.0405x over previous
"""Causal GQA attention (B=2, T=2048, C=2048, H=16, HKV=4, D=128, RoPE)
on 8 Trainium2 NeuronCores — v2.

Sharding: core c = (batch b = c//4, kv-group g = c%4): 4 q heads + 1 kv
head per core; row-parallel output projection, host sums 4 partials.

Design notes:
  - Projections run as fp8e4 hi/lo DoubleRow matmuls (3 compensation
    terms over k-tile pairs). x and w ship pre-split/pre-scaled; dequant
    folds into the RoPE tables (q/k), the v-copy scale, and the softmax
    scale applied at exp time.
  - bf16 storage elsewhere; fp8 only where noise is attenuated
    (off-diagonal softmax weights + hi/lo v).
  - Attention is block-causal at 128 granularity via suffix-truncated
    moving operands on the diagonal; off-diagonal s-tile pairs exp
    straight to fp8 and feed DoubleRow PV (v hi/lo) and DoubleRow
    ones-matmul denominators.
  - Output projection (bf16) is interleaved into phase B's instruction
    stream to fill the tensor engine during Act-bound stretches.
  - DMA pacing: packed projection-weight stream, wo deferred to phase
    B, trig tables per-chunk bf16 with pool-rotation gating, x
    triple-buffered.
  - RoPE: a partition-pre-swapped bf16 staging copy (Act; PSUM input
    permits the partition offset, SBUF x SBUF would not) lets the sin
    multiply run as one aligned 2-byte-mode DVE op.
"""

import os
from contextlib import ExitStack

import numpy as np

import concourse.bass as bass
import concourse.tile as tile
from concourse import bacc, mybir
from concourse.bass_utils import run_bass_kernel_spmd
from concourse.masks import make_identity

B, T, C = 2, 2048, 2048
H, HKV, D = 16, 4, 128
GROUP = H // HKV
THETA = 1000000.0
SCALE = D ** -0.5

P = 128
TCH = 512
NJT = T // TCH             # 4
NK = C // P                # 16 k-tiles
NKP = NK // 2              # 8 DoubleRow k-tile pairs
NH = GROUP                 # 4 local q heads
NST = T // P               # 16 s-tiles
N_CORES = 8

SX = 16.0                  # x fp8 scale
SW = 1024.0                # w fp8 scale (q/k/v)
SV = 2.0 ** -10            # v psum -> sbuf scale
CDEN = 0.25                # ones constant: ot = 64*O_true (fp8 range)
SWO = 1024.0               # wo fp8 scale
YDQ = 1.0 / (64.0 * SWO)   # host-side dequant of the y partials

F32 = mybir.dt.float32
BF16 = mybir.dt.bfloat16
F8 = mybir.dt.float8e4
DR = mybir.MatmulPerfMode.DoubleRow


def build_program(phases="ABC", variant=""):
    nc = bacc.Bacc("TRN2", target_bir_lowering=False, debug=False)

    xh_d = nc.dram_tensor("xh", [C, T], F8, kind="ExternalInput").ap()
    xl_d = nc.dram_tensor("xl", [C, T], F8, kind="ExternalInput").ap()
    WPK = NH * D + 2 * D       # packed row: wq 512 | wk 128 | wv 128
    wAh_d = nc.dram_tensor("wAh", [P, NKP * 2 * WPK], F8, kind="ExternalInput").ap()
    wAl_d = nc.dram_tensor("wAl", [P, NKP * 2 * WPK], F8, kind="ExternalInput").ap()
    woh_d = nc.dram_tensor("woh", [P, 2 * NJT * 2 * TCH], F8, kind="ExternalInput").ap()
    wol_d = nc.dram_tensor("wol", [P, 2 * NJT * 2 * TCH], F8, kind="ExternalInput").ap()
    cos_d = nc.dram_tensor("cosT", [P, T], BF16, kind="ExternalInput").ap()
    sin_d = nc.dram_tensor("sinT", [P, T], BF16, kind="ExternalInput").ap()
    tri_d = nc.dram_tensor("tri", [P, P], BF16, kind="ExternalInput").ap()
    on8_d = nc.dram_tensor("on8", [P, 256], F8, kind="ExternalInput").ap()
    onb_d = nc.dram_tensor("onb", [P, P], BF16, kind="ExternalInput").ap()
    y_d = nc.dram_tensor("y", [T, C], BF16, kind="ExternalOutput").ap()

    with tile.TileContext(nc) as tc, ExitStack() as ctx:
        wpool = ctx.enter_context(tc.tile_pool(name="weights", bufs=1))
        tpool = ctx.enter_context(tc.tile_pool(name="tables", bufs=1))
        trigp = ctx.enter_context(tc.tile_pool(name="trig", bufs=2))
        state = ctx.enter_context(tc.tile_pool(name="state", bufs=1))

        wAh_sb = wpool.tile([P, NKP, 2, WPK], F8, tag="wAh")
        wAl_sb = wpool.tile([P, NKP, 2, WPK], F8, tag="wAl")
        for h4 in range(4):
            nc.scalar.dma_start(
                wAh_sb[:, 2 * h4:2 * h4 + 2, :, :],
                wAh_d[:, 2 * h4 * 2 * WPK:(2 * h4 + 2) * 2 * WPK].rearrange(
                    "p (kp two o) -> p kp two o", kp=2, two=2))
        for h4 in range(4):
            nc.scalar.dma_start(
                wAl_sb[:, 2 * h4:2 * h4 + 2, :, :],
                wAl_d[:, 2 * h4 * 2 * WPK:(2 * h4 + 2) * 2 * WPK].rearrange(
                    "p (kp two o) -> p kp two o", kp=2, two=2))

        tri_sb = tpool.tile([P, P], BF16, tag="tri")
        nc.gpsimd.dma_start(tri_sb[:], tri_d[:])
        on8_sb = tpool.tile([P, 2, P], F8, tag="on8")
        nc.gpsimd.dma_start(on8_sb[:], on8_d.rearrange("p (two f) -> p two f", two=2))
        onb_sb = tpool.tile([P, P], BF16, tag="onb")
        nc.gpsimd.dma_start(onb_sb[:], onb_d[:])
        identb = tpool.tile([P, P], BF16, tag="ident")
        make_identity(nc, identb[:])
        # (variant parsed below, before use)

        # PE warmup: keep the tensor engine continuously busy from t~0 so the
        # p-state ramp completes before the first real (DMA-gated) matmuls.
        warmp = ctx.enter_context(tc.tile_pool(name="warm", bufs=1))
        warm_sb = warmp.tile([P, P], BF16, tag="w")
        nc.vector.memset(warm_sb[:], 0.0)
        nwarm = 0
        for v in variant.split("+"):
            if v.startswith("warm"):
                nwarm = int(v[4:])
        with tc.tile_pool(name="psW", bufs=1, space="PSUM") as psW:
            wp = psW.tile([P, P], F32, tag="wp")
            for i in range(nwarm):
                nc.tensor.matmul(wp[:], identb[:], warm_sb[:],
                                 start=(i == 0), stop=(i == nwarm - 1))

        qrot8 = state.tile([P, NH, 2, T], F8, tag="qrot8")
        krot8 = state.tile([P, 2, T], F8, tag="krot8")
        v_sb = state.tile([P, NST, D], BF16, tag="v")
        v8h = state.tile([P, NST // 2, 2, D], F8, tag="v8h")
        v8l = state.tile([P, NST // 2, 2, D], F8, tag="v8l")
        ot8h = state.tile([P, 2, NST, 2, D], F8, tag="ot8h")
        ot8l = state.tile([P, 2, NST, 2, D], F8, tag="ot8l")

        njt_lim = NJT
        for v in variant.split("+"):
            if v.startswith("njt"):
                njt_lim = int(v[3:])

        # ---------------- Phase A: projections + RoPE -----------------
        with ExitStack() as actx:
          if "A" in phases:
            xpool = actx.enter_context(tc.tile_pool(name="xsub", bufs=3))
            ropep = actx.enter_context(tc.tile_pool(name="rope", bufs=3))
            vtp = actx.enter_context(tc.tile_pool(name="vt", bufs=2))
            psA = actx.enter_context(tc.tile_pool(name="psA", bufs=1, space="PSUM"))
            psT = actx.enter_context(tc.tile_pool(name="psT", bufs=2, space="PSUM"))

            def rope(acc_ps, cos_t, sin_t):
                # partition-pre-swapped bf16 stage (Act, PSUM input permits the
                # offset) so both DVE multiplies run aligned in 2-byte mode
                absw = ropep.tile([P, TCH], BF16, tag="ab")
                nc.scalar.copy(absw[0:64, :], acc_ps[64:128, :])
                nc.scalar.copy(absw[64:128, :], acc_ps[0:64, :])
                m1 = ropep.tile([P, TCH], BF16, tag="m1")
                m2 = ropep.tile([P, TCH], BF16, tag="m2")
                nc.vector.tensor_tensor(
                    m1[:], acc_ps[:], cos_t[:], mybir.AluOpType.mult)
                nc.vector.tensor_tensor(
                    m2[:], absw[:], sin_t[:], mybir.AluOpType.mult)
                out = ropep.tile([P, TCH], BF16, tag="ro")
                nc.vector.tensor_tensor(
                    out[:], m1[:], m2[:], mybir.AluOpType.add)
                return out

            def w_slice(term, o, m):
                sb = wAh_sb if term != 2 else wAl_sb
                if o < NH:
                    return sb[:, m, :, o * D:(o + 1) * D]
                if o == NH:
                    return sb[:, m, :, NH * D:NH * D + D]
                return sb[:, m, :, NH * D + D:NH * D + 2 * D]

            def finish(o, acc, jt, cos_t, sin_t):
                """Returns a deferred-emission thunk (or None)."""
                if "noropeA" in variant:
                    return None
                ch_ = slice(jt * TCH, (jt + 1) * TCH)
                if o < NH:
                    qt = rope(acc, cos_t, sin_t)
                    qh_ap = qrot8[:, o, 0, ch_]
                    nc.vector.tensor_copy(qh_ap, qt[:])
                    nc.vector.tensor_tensor(
                        qrot8[:, o, 1, ch_], qt[:], qh_ap,
                        mybir.AluOpType.subtract)
                    return None
                if o == NH:
                    kt = rope(acc, cos_t, sin_t)
                    nc.scalar.copy(krot8[:, 0, ch_], kt[:])
                    nc.scalar.copy(krot8[:, 1, ch_], kt[:])
                    return None
                vt = vtp.tile([P, TCH], BF16, tag="vt")
                nc.scalar.activation(
                    vt[:], acc[:], mybir.ActivationFunctionType.Copy, scale=SV)

                def transposes():
                    for i in range(TCH // P):
                        s_idx = jt * (TCH // P) + i
                        pst = psT.tile([P, P], BF16, tag="pst")
                        nc.tensor.transpose(pst[:], vt[:, i * P:(i + 1) * P],
                                            identb[:])
                        nc.scalar.copy(v_sb[:, s_idx, :], pst[:])
                        nc.scalar.copy(v8h[:, s_idx // 2, s_idx % 2, :], pst[:])
                        nc.vector.tensor_tensor(
                            v8l[:, s_idx // 2, s_idx % 2, :], pst[:],
                            v8h[:, s_idx // 2, s_idx % 2, :],
                            mybir.AluOpType.subtract)
                return transposes

            nacc = 0
            deferred = None
            for jt in range(njt_lim):
                cos_t = trigp.tile([P, TCH], BF16, tag="cos")
                sin_t = trigp.tile([P, TCH], BF16, tag="sin")
                ch = slice(jt * TCH, (jt + 1) * TCH)
                nc.gpsimd.dma_start(cos_t[:], cos_d[:, ch])
                nc.gpsimd.dma_start(sin_t[:], sin_d[:, ch])

                xhs, xls = [], []
                for m in range(NKP):
                    xt = xpool.tile([P, 2, TCH], F8, tag=f"xh{m}")
                    nc.sync.dma_start(
                        xt[:],
                        xh_d[2 * m * P:(2 * m + 2) * P, ch].rearrange(
                            "(two p) t -> p two t", p=P))
                    xhs.append(xt)
                for m in range(NKP):
                    xt = xpool.tile([P, 2, TCH], F8, tag=f"xl{m}")
                    nc.sync.dma_start(
                        xt[:],
                        xl_d[2 * m * P:(2 * m + 2) * P, ch].rearrange(
                            "(two p) t -> p two t", p=P))
                    xls.append(xt)

                if jt == 0:
                    accs = [psA.tile([P, TCH], F32, tag=f"acc{o}",
                                     name=f"acc{o}")
                            for o in range(6)]
                    for ti, (xs, term) in enumerate(((xhs, 0), (xls, 1),
                                                     (xhs, 2))):
                        for m in range(NKP):
                            for o in (4, 5, 0, 1, 2, 3):
                                nc.tensor.matmul(
                                    accs[o][:], w_slice(term, o, m),
                                    xs[m][:],
                                    start=(ti == 0 and m == 0),
                                    stop=(ti == 2 and m == NKP - 1),
                                    perf_mode=DR)
                    for o in (5, 4, 0, 1, 2, 3):
                        d = finish(o, accs[o], jt, cos_t, sin_t)
                        if d is not None:
                            deferred = d
                    nacc = 5
                else:
                    for o in (4, 5, 0, 1, 2, 3):
                        acc = psA.tile([P, TCH], F32, tag=f"acc{nacc % 6}",
                                       name=f"accr{nacc % 6}")
                        nacc += 1
                        first = True
                        for xs, term in (((xhs, 0), (xls, 1), (xhs, 2))):
                            for m in range(NKP):
                                nc.tensor.matmul(
                                    acc[:], w_slice(term, o, m), xs[m][:],
                                    start=first,
                                    stop=(term == 2 and m == NKP - 1),
                                    perf_mode=DR)
                                first = False
                        if deferred is not None:
                            deferred()
                            deferred = None
                        d = finish(o, acc, jt, cos_t, sin_t)
                        if d is not None:
                            deferred = d
                if jt == njt_lim - 1 and deferred is not None:
                    deferred()
                    deferred = None

        # ---------------- Phase B + C interleaved ----------------------
        with ExitStack() as bctx:
          if "B" in phases:
            e8p = bctx.enter_context(tc.tile_pool(name="e8", bufs=6))
            edp = bctx.enter_context(tc.tile_pool(name="ed", bufs=3))
            rcp = bctx.enter_context(tc.tile_pool(name="rc", bufs=3))
            wopool = bctx.enter_context(tc.tile_pool(name="wo", bufs=1))
            ypool = bctx.enter_context(tc.tile_pool(name="ysb", bufs=6))
            psS = bctx.enter_context(tc.tile_pool(name="psS", bufs=2, space="PSUM"))
            psO = bctx.enter_context(tc.tile_pool(name="psO", bufs=1, space="PSUM"))
            psD = bctx.enter_context(tc.tile_pool(name="psD", bufs=1, space="PSUM"))
            psC = bctx.enter_context(tc.tile_pool(name="psC", bufs=2, space="PSUM"))

            woh_sb = wopool.tile([P, 2, NJT, 2, TCH], F8, tag="woh")
            wol_sb = wopool.tile([P, 2, NJT, 2, TCH], F8, tag="wol")
            nc.sync.dma_start(
                woh_sb[:], woh_d.rearrange("p (i jc two t) -> p i jc two t",
                                           i=2, jc=NJT, two=2))
            nc.sync.dma_start(
                wol_sb[:], wol_d.rearrange("p (i jc two t) -> p i jc two t",
                                           i=2, jc=NJT, two=2))

            def c_group(tt, jc):
                yp = psC.tile([P, TCH], F32, tag="y")
                first = True
                for i in range(2):
                    for lhs, rhs in ((ot8h, woh_sb), (ot8l, woh_sb),
                                     (ot8h, wol_sb)):
                        nc.tensor.matmul(
                            yp[:], lhs[:, i, tt, :, :], rhs[:, i, jc, :, :],
                            start=first, stop=(i == 1 and rhs is wol_sb),
                            perf_mode=DR, skip_group_check=True)
                        first = False
                ys = ypool.tile([P, TCH], BF16, tag="ys")
                nc.vector.tensor_copy(ys[:], yp[:])
                nc.sync.dma_start(
                    y_d[tt * P:(tt + 1) * P, jc * TCH:(jc + 1) * TCH],
                    ys[:])

            do_c = "C" in phases

            nbjt = NJT
            for v in variant.split("+"):
                if v.startswith("nbjt"):
                    nbjt = int(v[4:])
            cpend = []

            def emit_c(n=1):
                for _ in range(n):
                    if cpend:
                        c_group(*cpend.pop(0))

            for jt in range(nbjt):
                ch = slice(jt * TCH, (jt + 1) * TCH)
                if do_c and jt >= 1:
                    cpend = [(4 * (jt - 1) + tt4, jc)
                             for tt4 in range(4) for jc in range(NJT)]
                for h in range(NH):
                    qch = qrot8[:, h, :, ch]
                    ot_ps = psO.tile([P, TCH], F32, tag="ot")
                    dn_ps = psD.tile([P, TCH], F32, tag="dn")
                    npair = 2 * jt

                    def sc_pair(m):
                        sps = psS.tile([P, 2 * TCH], F32, tag="su")
                        for i in range(2):
                            js = 2 * m + i
                            nc.tensor.matmul(
                                sps[:, i * TCH:(i + 1) * TCH],
                                krot8[:, :, js * P:(js + 1) * P], qch,
                                start=True, stop=True, perf_mode=DR,
                                skip_group_check=True)
                        e8 = e8p.tile([P, 2, TCH], F8, tag="e8")
                        nc.scalar.activation(
                            e8[:], sps[:], mybir.ActivationFunctionType.Exp,
                            scale=SCALE / 256.0)
                        return e8

                    def pv_pair(m, e8, start):
                        nc.tensor.matmul(
                            ot_ps[:], v8h[:, m, :, :], e8[:],
                            start=start, stop=False, perf_mode=DR,
                            skip_group_check=True)
                        nc.tensor.matmul(
                            ot_ps[:], v8l[:, m, :, :], e8[:],
                            start=False, stop=False, perf_mode=DR,
                            skip_group_check=True)
                        nc.tensor.matmul(
                            dn_ps[:], on8_sb[:], e8[:],
                            start=start, stop=False, perf_mode=DR,
                            skip_group_check=True)

                    pend = []
                    for m in range(npair):
                        e8 = sc_pair(m)
                        if len(pend) >= 4:
                            mm, ee = pend.pop(0)
                            pv_pair(mm, ee, mm == 0)
                        pend.append((m, e8))

                    dsup = []
                    for half in range(2):
                        sps = psS.tile([P, 2 * TCH], F32, tag="su")
                        ed = edp.tile([P, 2 * TCH], BF16, tag="ed")
                        widths = []
                        off = 0
                        for rr in range(2):
                            r = 2 * half + rr
                            w = TCH - r * P
                            js = 4 * jt + r
                            nc.tensor.matmul(
                                sps[:, off:off + w],
                                krot8[:, :, js * P:(js + 1) * P],
                                qch[:, :, r * P:],
                                start=True, stop=True, perf_mode=DR,
                                skip_group_check=True)
                            widths.append((r, off, w))
                            off += w
                        nc.scalar.activation(
                            ed[:, 0:off], sps[:, 0:off],
                            mybir.ActivationFunctionType.Exp, scale=SCALE / 256.0)
                        for r, off_, w in widths:
                            nc.vector.tensor_tensor(
                                ed[:, off_:off_ + P], ed[:, off_:off_ + P],
                                tri_sb[:], mybir.AluOpType.mult)
                        dsup.append((ed, widths))
                        while pend:
                            mm, ee = pend.pop(0)
                            pv_pair(mm, ee, mm == 0)

                    for ed, widths in dsup:
                        for r, off, w in widths:
                            js = 4 * jt + r
                            nc.tensor.matmul(
                                ot_ps[:, r * P:], v_sb[:, js, :],
                                ed[:, off:off + w],
                                start=(jt == 0 and r == 0), stop=(r == 3),
                                skip_group_check=True)
                            nc.tensor.matmul(
                                dn_ps[:, r * P:], onb_sb[:],
                                ed[:, off:off + w],
                                start=(jt == 0 and r == 0), stop=(r == 3),
                                skip_group_check=True)

                    if not (jt == NJT - 1 and h == NH - 1):
                        emit_c(4)
                    rb = rcp.tile([P, TCH], F32, tag="rb")
                    nc.vector.reciprocal(rb[:], dn_ps[:])
                    tmp = rcp.tile([P, 4, P], F32, tag="tmp")
                    nc.vector.tensor_tensor(
                        tmp[:].rearrange("p a b -> p (a b)"), ot_ps[:], rb[:],
                        mybir.AluOpType.mult)
                    oh = ot8h[:, h // 2, 4 * jt:4 * jt + 4, h % 2, :]
                    ol = ot8l[:, h // 2, 4 * jt:4 * jt + 4, h % 2, :]
                    nc.vector.tensor_copy(oh, tmp[:])
                    nc.vector.tensor_tensor(
                        ol, tmp[:], oh, mybir.AluOpType.subtract)
                    if jt == NJT - 1 and h == NH - 1:
                        emit_c(4)


            if do_c:
                for tt4 in range(4):
                    for jc in range(NJT):
                        c_group(12 + tt4, jc)

    nc.compile()
    return nc


def host_prep(x, wq, wk, wv, wo):
    import ml_dtypes
    F8np = ml_dtypes.float8_e4m3
    BFnp = ml_dtypes.bfloat16

    x = np.asarray(x, dtype=np.float32)
    wq = np.asarray(wq, dtype=np.float32)
    wk = np.asarray(wk, dtype=np.float32)
    wv = np.asarray(wv, dtype=np.float32)
    wo = np.asarray(wo, dtype=np.float32)

    perm = np.concatenate([np.arange(0, D, 2), np.arange(1, D, 2)])

    inv_freq = (1.0 / THETA ** (np.arange(0, D, 2, dtype=np.float32) / D)).astype(np.float32)
    pos = np.arange(T, dtype=np.float32)
    freqs = pos[:, None] * inv_freq[None, :]
    cos_t = np.cos(freqs).astype(np.float32).T
    sin_t = np.sin(freqs).astype(np.float32).T
    dq = np.float32(16.0 / (SX * SW))
    cosT = np.concatenate([cos_t, cos_t], axis=0) * dq
    sinT = np.concatenate([-sin_t, sin_t], axis=0) * dq

    tri = (np.arange(P)[None, :] >= np.arange(P)[:, None]).astype(BFnp)

    def hilo(a):
        h = a.astype(F8np)
        l = (a - h.astype(np.float32)).astype(F8np)
        return h, l

    xs = [np.ascontiguousarray(x[b].T) * SX for b in range(B)]
    xhl = [hilo(a) for a in xs]

    in_maps = []
    for c in range(N_CORES):
        b, g = divmod(c, GROUP)
        rows = []
        for hh in range(NH):
            h = g * GROUP + hh
            rows.append(wq[h * D + perm, :])
        wq_g = np.concatenate(rows, axis=0) * SW          # [512, C]
        wk_g = wk[g * D + perm, :] * SW
        wv_g = wv[g * D:(g + 1) * D, :] * SW
        wo_g = wo[:, g * NH * D:(g + 1) * NH * D]         # [C, 512]

        wqT = np.ascontiguousarray(wq_g.T)                # [C, 512]
        wkT = np.ascontiguousarray(wk_g.T)                # [C, 128]
        wvT = np.ascontiguousarray(wv_g.T)                # [C, 128]
        rows = np.arange(C).reshape(NKP, 2, P)            # [kp, two, p]
        pk = np.concatenate([wqT[rows], wkT[rows], wvT[rows]], axis=-1)
        pk = np.ascontiguousarray(np.transpose(pk, (2, 0, 1, 3)))  # [p,kp,two,col]
        wAh, wAl = hilo(pk.reshape(P, -1))
        # wo pair-contiguous DR layout: [d, hpair, jc, h-in-pair, tc]
        woT = np.ascontiguousarray(wo_g.T) * SWO          # [512, C]
        wo_a = woT.reshape(2, 2, D, NJT, TCH)             # [i, hh, d, jc, tc]
        wo_b = np.ascontiguousarray(np.transpose(wo_a, (2, 0, 3, 1, 4)))
        woh, wol = hilo(wo_b.reshape(P, -1))

        in_maps.append({
            "xh": xhl[b][0], "xl": xhl[b][1],
            "wAh": wAh, "wAl": wAl,
            "woh": woh, "wol": wol,
            "cosT": cosT.astype(BFnp),
            "sinT": sinT.astype(BFnp),
            "tri": tri,
            "on8": np.full((P, 256), CDEN, dtype=F8np),
            "onb": np.full((P, P), CDEN, dtype=BFnp),
        })
    return in_maps


_CACHE = {}


def _get_program(key="v2"):
    if key not in _CACHE:
        _CACHE[key] = build_program()
    return _CACHE[key]


def kernel(x, mask, wq, wk, wv, wo):
    nc = _get_program()
    in_maps = host_prep(x, wq, wk, wv, wo)
    res = run_bass_kernel_spmd(nc, in_maps, list(range(N_CORES))).results
    out = np.zeros((B, T, C), dtype=np.float32)
    for c in range(N_CORES):
        out[c // GROUP] += res[c]["y"].astype(np.float32) * YDQ
    return out


# revision 12
# speedup vs baseline: 1.0538x; 1.0128x over previous
"""Causal GQA attention (B=2, T=2048, C=2048, H=16, HKV=4, D=128, RoPE)
on 8 Trainium2 NeuronCores — v2.

Sharding: core c = (batch b = c//4, kv-group g = c%4): 4 q heads + 1 kv
head per core; row-parallel output projection, host sums 4 partials.

Design notes:
  - Projections run as fp8e4 hi/lo DoubleRow matmuls (3 compensation
    terms over k-tile pairs). x and w ship pre-split/pre-scaled; dequant
    folds into the RoPE tables (q/k), the v-copy scale, and the softmax
    scale applied at exp time.
  - Attention scores also run as DoubleRow: k plain fp8 (scaled x16,
    duplicated for the stationary pair) against q in hi/lo fp8 — the two
    compensation products pack into one DoubleRow instruction, halving
    score-matmul cost. Verified end-to-end error 1.13e-2 vs the 2e-2
    gate. Remaining bf16: diagonal softmax tiles only.
  - Attention is block-causal at 128 granularity via suffix-truncated
    moving operands on the diagonal; off-diagonal s-tile pairs exp
    straight to fp8 and feed DoubleRow PV (v hi/lo) and DoubleRow
    ones-matmul denominators.
  - Output projection (bf16) is interleaved into phase B's instruction
    stream to fill the tensor engine during Act-bound stretches.
  - DMA pacing: packed projection-weight stream, wo deferred to phase
    B, trig tables per-chunk bf16 with pool-rotation gating, x
    triple-buffered.
  - RoPE: a partition-pre-swapped bf16 staging copy (Act; PSUM input
    permits the partition offset, SBUF x SBUF would not) lets the sin
    multiply run as one aligned 2-byte-mode DVE op.
"""

import os
from contextlib import ExitStack

import numpy as np

import concourse.bass as bass
import concourse.tile as tile
from concourse import bacc, mybir
from concourse.bass_utils import run_bass_kernel_spmd
from concourse.masks import make_identity

B, T, C = 2, 2048, 2048
H, HKV, D = 16, 4, 128
GROUP = H // HKV
THETA = 1000000.0
SCALE = D ** -0.5

P = 128
TCH = 512
NJT = T // TCH             # 4
NK = C // P                # 16 k-tiles
NKP = NK // 2              # 8 DoubleRow k-tile pairs
NH = GROUP                 # 4 local q heads
NST = T // P               # 16 s-tiles
N_CORES = 8

SX = 16.0                  # x fp8 scale
SW = 1024.0                # w fp8 scale (q/k/v)
SV = 2.0 ** -10            # v psum -> sbuf scale
CDEN = 0.25                # ones constant: ot = 64*O_true (fp8 range)
SWO = 1024.0               # wo fp8 scale
YDQ = 1.0 / (64.0 * SWO)   # host-side dequant of the y partials

F32 = mybir.dt.float32
BF16 = mybir.dt.bfloat16
F8 = mybir.dt.float8e4
DR = mybir.MatmulPerfMode.DoubleRow


def build_program(phases="ABC", variant=""):
    nc = bacc.Bacc("TRN2", target_bir_lowering=False, debug=False)

    xh_d = nc.dram_tensor("xh", [C, T], F8, kind="ExternalInput").ap()
    xl_d = nc.dram_tensor("xl", [C, T], F8, kind="ExternalInput").ap()
    WPK = NH * D + 2 * D       # packed row: wq 512 | wk 128 | wv 128
    wAh_d = nc.dram_tensor("wAh", [P, NKP * 2 * WPK], F8, kind="ExternalInput").ap()
    wAl_d = nc.dram_tensor("wAl", [P, NKP * 2 * WPK], F8, kind="ExternalInput").ap()
    woh_d = nc.dram_tensor("woh", [P, 2 * NJT * 2 * TCH], F8, kind="ExternalInput").ap()
    wol_d = nc.dram_tensor("wol", [P, 2 * NJT * 2 * TCH], F8, kind="ExternalInput").ap()
    cos_d = nc.dram_tensor("cosT", [P, T], BF16, kind="ExternalInput").ap()
    sin_d = nc.dram_tensor("sinT", [P, T], BF16, kind="ExternalInput").ap()
    tri_d = nc.dram_tensor("tri", [P, P], BF16, kind="ExternalInput").ap()
    on8_d = nc.dram_tensor("on8", [P, 256], F8, kind="ExternalInput").ap()
    onb_d = nc.dram_tensor("onb", [P, P], BF16, kind="ExternalInput").ap()
    y_d = nc.dram_tensor("y", [T, C], BF16, kind="ExternalOutput").ap()

    with tile.TileContext(nc) as tc, ExitStack() as ctx:
        wpool = ctx.enter_context(tc.tile_pool(name="weights", bufs=1))
        tpool = ctx.enter_context(tc.tile_pool(name="tables", bufs=1))
        trigp = ctx.enter_context(tc.tile_pool(name="trig", bufs=2))
        state = ctx.enter_context(tc.tile_pool(name="state", bufs=1))

        wAh_sb = wpool.tile([P, NKP, 2, WPK], F8, tag="wAh")
        wAl_sb = wpool.tile([P, NKP, 2, WPK], F8, tag="wAl")
        for h4 in range(4):
            nc.scalar.dma_start(
                wAh_sb[:, 2 * h4:2 * h4 + 2, :, :],
                wAh_d[:, 2 * h4 * 2 * WPK:(2 * h4 + 2) * 2 * WPK].rearrange(
                    "p (kp two o) -> p kp two o", kp=2, two=2))
        for h4 in range(4):
            nc.scalar.dma_start(
                wAl_sb[:, 2 * h4:2 * h4 + 2, :, :],
                wAl_d[:, 2 * h4 * 2 * WPK:(2 * h4 + 2) * 2 * WPK].rearrange(
                    "p (kp two o) -> p kp two o", kp=2, two=2))

        tri_sb = tpool.tile([P, P], BF16, tag="tri")
        nc.gpsimd.dma_start(tri_sb[:], tri_d[:])
        on8_sb = tpool.tile([P, 2, P], F8, tag="on8")
        nc.gpsimd.dma_start(on8_sb[:], on8_d.rearrange("p (two f) -> p two f", two=2))
        onb_sb = tpool.tile([P, P], BF16, tag="onb")
        nc.gpsimd.dma_start(onb_sb[:], onb_d[:])
        identb = tpool.tile([P, P], BF16, tag="ident")
        make_identity(nc, identb[:])
        # (variant parsed below, before use)

        # PE warmup: keep the tensor engine continuously busy from t~0 so the
        # p-state ramp completes before the first real (DMA-gated) matmuls.
        warmp = ctx.enter_context(tc.tile_pool(name="warm", bufs=1))
        warm_sb = warmp.tile([P, P], BF16, tag="w")
        nc.vector.memset(warm_sb[:], 0.0)
        nwarm = 0
        for v in variant.split("+"):
            if v.startswith("warm"):
                nwarm = int(v[4:])
        with tc.tile_pool(name="psW", bufs=1, space="PSUM") as psW:
            wp = psW.tile([P, P], F32, tag="wp")
            for i in range(nwarm):
                nc.tensor.matmul(wp[:], identb[:], warm_sb[:],
                                 start=(i == 0), stop=(i == nwarm - 1))

        qrot8 = state.tile([P, NH, 2, T], F8, tag="qrot8")
        krot8 = state.tile([P, 2, T], F8, tag="krot8")
        v_sb = state.tile([P, NST, D], BF16, tag="v")
        v8h = state.tile([P, NST // 2, 2, D], F8, tag="v8h")
        v8l = state.tile([P, NST // 2, 2, D], F8, tag="v8l")
        ot8h = state.tile([P, 2, NST, 2, D], F8, tag="ot8h")
        ot8l = state.tile([P, 2, NST, 2, D], F8, tag="ot8l")

        njt_lim = NJT
        for v in variant.split("+"):
            if v.startswith("njt"):
                njt_lim = int(v[3:])

        # ---------------- Phase A: projections + RoPE -----------------
        with ExitStack() as actx:
          if "A" in phases:
            xpool = actx.enter_context(tc.tile_pool(name="xsub", bufs=3))
            ropep = actx.enter_context(tc.tile_pool(name="rope", bufs=3))
            vtp = actx.enter_context(tc.tile_pool(name="vt", bufs=2))
            psA = actx.enter_context(tc.tile_pool(name="psA", bufs=1, space="PSUM"))
            psT = actx.enter_context(tc.tile_pool(name="psT", bufs=2, space="PSUM"))

            def rope(acc_ps, cos_t, sin_t):
                # partition-pre-swapped bf16 stage (Act, PSUM input permits the
                # offset) so both DVE multiplies run aligned in 2-byte mode
                absw = ropep.tile([P, TCH], BF16, tag="ab")
                nc.scalar.copy(absw[0:64, :], acc_ps[64:128, :])
                nc.scalar.copy(absw[64:128, :], acc_ps[0:64, :])
                m1 = ropep.tile([P, TCH], BF16, tag="m1")
                m2 = ropep.tile([P, TCH], BF16, tag="m2")
                nc.vector.tensor_tensor(
                    m1[:], acc_ps[:], cos_t[:], mybir.AluOpType.mult)
                nc.vector.tensor_tensor(
                    m2[:], absw[:], sin_t[:], mybir.AluOpType.mult)
                out = ropep.tile([P, TCH], BF16, tag="ro")
                nc.vector.tensor_tensor(
                    out[:], m1[:], m2[:], mybir.AluOpType.add)
                return out

            def w_slice(term, o, m):
                sb = wAh_sb if term != 2 else wAl_sb
                if o < NH:
                    return sb[:, m, :, o * D:(o + 1) * D]
                if o == NH:
                    return sb[:, m, :, NH * D:NH * D + D]
                return sb[:, m, :, NH * D + D:NH * D + 2 * D]

            def finish(o, acc, jt, cos_t, sin_t):
                """Returns a deferred-emission thunk (or None)."""
                if "noropeA" in variant:
                    return None
                ch_ = slice(jt * TCH, (jt + 1) * TCH)
                if o < NH:
                    qt = rope(acc, cos_t, sin_t)
                    qh_ap = qrot8[:, o, 0, ch_]
                    nc.vector.tensor_copy(qh_ap, qt[:])
                    nc.vector.tensor_tensor(
                        qrot8[:, o, 1, ch_], qt[:], qh_ap,
                        mybir.AluOpType.subtract)
                    return None
                if o == NH:
                    kt = rope(acc, cos_t, sin_t)
                    nc.scalar.copy(krot8[:, 0, ch_], kt[:])
                    nc.scalar.copy(krot8[:, 1, ch_], kt[:])
                    return None
                vt = vtp.tile([P, TCH], BF16, tag="vt")
                nc.scalar.activation(
                    vt[:], acc[:], mybir.ActivationFunctionType.Copy, scale=SV)

                def transposes():
                    for i in range(TCH // P):
                        s_idx = jt * (TCH // P) + i
                        pst = psT.tile([P, P], BF16, tag="pst")
                        nc.tensor.transpose(pst[:], vt[:, i * P:(i + 1) * P],
                                            identb[:])
                        nc.scalar.copy(v_sb[:, s_idx, :], pst[:])
                        nc.scalar.copy(v8h[:, s_idx // 2, s_idx % 2, :], pst[:])
                        nc.vector.tensor_tensor(
                            v8l[:, s_idx // 2, s_idx % 2, :], pst[:],
                            v8h[:, s_idx // 2, s_idx % 2, :],
                            mybir.AluOpType.subtract)
                return transposes

            nacc = 0
            deferred = None
            for jt in range(njt_lim):
                cos_t = trigp.tile([P, TCH], BF16, tag="cos")
                sin_t = trigp.tile([P, TCH], BF16, tag="sin")
                ch = slice(jt * TCH, (jt + 1) * TCH)
                nc.gpsimd.dma_start(cos_t[:], cos_d[:, ch])
                nc.gpsimd.dma_start(sin_t[:], sin_d[:, ch])

                xhs, xls = [], []
                for m in range(NKP):
                    xt = xpool.tile([P, 2, TCH], F8, tag=f"xh{m}")
                    nc.sync.dma_start(
                        xt[:],
                        xh_d[2 * m * P:(2 * m + 2) * P, ch].rearrange(
                            "(two p) t -> p two t", p=P))
                    xhs.append(xt)
                for m in range(NKP):
                    xt = xpool.tile([P, 2, TCH], F8, tag=f"xl{m}")
                    nc.sync.dma_start(
                        xt[:],
                        xl_d[2 * m * P:(2 * m + 2) * P, ch].rearrange(
                            "(two p) t -> p two t", p=P))
                    xls.append(xt)

                if jt == 0:
                    accs = [psA.tile([P, TCH], F32, tag=f"acc{o}",
                                     name=f"acc{o}")
                            for o in range(6)]
                    for ti, (xs, term) in enumerate(((xhs, 0), (xls, 1),
                                                     (xhs, 2))):
                        for m in range(NKP):
                            for o in (4, 5, 0, 1, 2, 3):
                                nc.tensor.matmul(
                                    accs[o][:], w_slice(term, o, m),
                                    xs[m][:],
                                    start=(ti == 0 and m == 0),
                                    stop=(ti == 2 and m == NKP - 1),
                                    perf_mode=DR)
                    for o in (5, 4, 0, 1, 2, 3):
                        d = finish(o, accs[o], jt, cos_t, sin_t)
                        if d is not None:
                            deferred = d
                    nacc = 5
                else:
                    for o in (4, 5, 0, 1, 2, 3):
                        acc = psA.tile([P, TCH], F32, tag=f"acc{nacc % 6}",
                                       name=f"accr{nacc % 6}")
                        nacc += 1
                        first = True
                        for xs, term in (((xhs, 0), (xls, 1), (xhs, 2))):
                            for m in range(NKP):
                                nc.tensor.matmul(
                                    acc[:], w_slice(term, o, m), xs[m][:],
                                    start=first,
                                    stop=(term == 2 and m == NKP - 1),
                                    perf_mode=DR)
                                first = False
                        if deferred is not None:
                            deferred()
                            deferred = None
                        d = finish(o, acc, jt, cos_t, sin_t)
                        if d is not None:
                            deferred = d
                if jt == njt_lim - 1 and deferred is not None:
                    deferred()
                    deferred = None

        # ---------------- Phase B + C interleaved ----------------------
        with ExitStack() as bctx:
          if "B" in phases:
            e8p = bctx.enter_context(tc.tile_pool(name="e8", bufs=6))
            edp = bctx.enter_context(tc.tile_pool(name="ed", bufs=3))
            rcp = bctx.enter_context(tc.tile_pool(name="rc", bufs=3))
            wopool = bctx.enter_context(tc.tile_pool(name="wo", bufs=1))
            ypool = bctx.enter_context(tc.tile_pool(name="ysb", bufs=6))
            psS = bctx.enter_context(tc.tile_pool(name="psS", bufs=2, space="PSUM"))
            psO = bctx.enter_context(tc.tile_pool(name="psO", bufs=1, space="PSUM"))
            psD = bctx.enter_context(tc.tile_pool(name="psD", bufs=1, space="PSUM"))
            psC = bctx.enter_context(tc.tile_pool(name="psC", bufs=2, space="PSUM"))

            woh_sb = wopool.tile([P, 2, NJT, 2, TCH], F8, tag="woh")
            wol_sb = wopool.tile([P, 2, NJT, 2, TCH], F8, tag="wol")
            nc.sync.dma_start(
                woh_sb[:], woh_d.rearrange("p (i jc two t) -> p i jc two t",
                                           i=2, jc=NJT, two=2))
            nc.sync.dma_start(
                wol_sb[:], wol_d.rearrange("p (i jc two t) -> p i jc two t",
                                           i=2, jc=NJT, two=2))

            def c_group(tt, jc):
                yp = psC.tile([P, TCH], F32, tag="y")
                first = True
                for i in range(2):
                    for lhs, rhs in ((ot8h, woh_sb), (ot8l, woh_sb),
                                     (ot8h, wol_sb)):
                        nc.tensor.matmul(
                            yp[:], lhs[:, i, tt, :, :], rhs[:, i, jc, :, :],
                            start=first, stop=(i == 1 and rhs is wol_sb),
                            perf_mode=DR, skip_group_check=True)
                        first = False
                ys = ypool.tile([P, TCH], BF16, tag="ys")
                nc.vector.tensor_copy(ys[:], yp[:])
                nc.sync.dma_start(
                    y_d[tt * P:(tt + 1) * P, jc * TCH:(jc + 1) * TCH],
                    ys[:])

            do_c = "C" in phases

            nbjt = NJT
            for v in variant.split("+"):
                if v.startswith("nbjt"):
                    nbjt = int(v[4:])
            cpend = []

            def emit_c(n=1):
                for _ in range(n):
                    if cpend:
                        c_group(*cpend.pop(0))

            for jt in range(nbjt):
                ch = slice(jt * TCH, (jt + 1) * TCH)
                if do_c and jt >= 1:
                    cpend = [(4 * (jt - 1) + tt4, jc)
                             for tt4 in range(4) for jc in range(NJT)]
                for h in range(NH):
                    qch = qrot8[:, h, :, ch]
                    ot_ps = psO.tile([P, TCH], F32, tag="ot")
                    dn_ps = psD.tile([P, TCH], F32, tag="dn")
                    npair = 2 * jt

                    def sc_pair(m):
                        sps = psS.tile([P, 2 * TCH], F32, tag="su")
                        for i in range(2):
                            js = 2 * m + i
                            nc.tensor.matmul(
                                sps[:, i * TCH:(i + 1) * TCH],
                                krot8[:, :, js * P:(js + 1) * P], qch,
                                start=True, stop=True, perf_mode=DR,
                                skip_group_check=True)
                        e8 = e8p.tile([P, 2, TCH], F8, tag="e8")
                        nc.scalar.activation(
                            e8[:], sps[:], mybir.ActivationFunctionType.Exp,
                            scale=SCALE / 256.0)
                        return e8

                    def pv_pair(m, e8, start):
                        nc.tensor.matmul(
                            ot_ps[:], v8h[:, m, :, :], e8[:],
                            start=start, stop=False, perf_mode=DR,
                            skip_group_check=True)
                        nc.tensor.matmul(
                            ot_ps[:], v8l[:, m, :, :], e8[:],
                            start=False, stop=False, perf_mode=DR,
                            skip_group_check=True)
                        nc.tensor.matmul(
                            dn_ps[:], on8_sb[:], e8[:],
                            start=start, stop=False, perf_mode=DR,
                            skip_group_check=True)

                    pend = []
                    for m in range(npair):
                        e8 = sc_pair(m)
                        if len(pend) >= 4:
                            mm, ee = pend.pop(0)
                            pv_pair(mm, ee, mm == 0)
                        pend.append((m, e8))

                    dsup = []
                    for half in range(2):
                        sps = psS.tile([P, 2 * TCH], F32, tag="su")
                        ed = edp.tile([P, 2 * TCH], BF16, tag="ed")
                        widths = []
                        off = 0
                        for rr in range(2):
                            r = 2 * half + rr
                            w = TCH - r * P
                            js = 4 * jt + r
                            nc.tensor.matmul(
                                sps[:, off:off + w],
                                krot8[:, :, js * P:(js + 1) * P],
                                qch[:, :, r * P:],
                                start=True, stop=True, perf_mode=DR,
                                skip_group_check=True)
                            widths.append((r, off, w))
                            off += w
                        nc.scalar.activation(
                            ed[:, 0:off], sps[:, 0:off],
                            mybir.ActivationFunctionType.Exp, scale=SCALE / 256.0)
                        for r, off_, w in widths:
                            nc.vector.tensor_tensor(
                                ed[:, off_:off_ + P], ed[:, off_:off_ + P],
                                tri_sb[:], mybir.AluOpType.mult)
                        dsup.append((ed, widths))
                        while pend:
                            mm, ee = pend.pop(0)
                            pv_pair(mm, ee, mm == 0)

                    for ed, widths in dsup:
                        for r, off, w in widths:
                            js = 4 * jt + r
                            nc.tensor.matmul(
                                ot_ps[:, r * P:], v_sb[:, js, :],
                                ed[:, off:off + w],
                                start=(jt == 0 and r == 0), stop=(r == 3),
                                skip_group_check=True)
                            nc.tensor.matmul(
                                dn_ps[:, r * P:], onb_sb[:],
                                ed[:, off:off + w],
                                start=(jt == 0 and r == 0), stop=(r == 3),
                                skip_group_check=True)

                    if not (jt == NJT - 1 and h == NH - 1):
                        emit_c(4)
                    rb = rcp.tile([P, TCH], F32, tag="rb")
                    nc.vector.reciprocal(rb[:], dn_ps[:])
                    tmp = rcp.tile([P, 4, P], F32, tag="tmp")
                    nc.vector.tensor_tensor(
                        tmp[:].rearrange("p a b -> p (a b)"), ot_ps[:], rb[:],
                        mybir.AluOpType.mult)
                    oh = ot8h[:, h // 2, 4 * jt:4 * jt + 4, h % 2, :]
                    ol = ot8l[:, h // 2, 4 * jt:4 * jt + 4, h % 2, :]
                    nc.vector.tensor_copy(oh, tmp[:])
                    nc.vector.tensor_tensor(
                        ol, tmp[:], oh, mybir.AluOpType.subtract)
                    if jt == NJT - 1 and h == NH - 1:
                        emit_c(4)


            if do_c:
                for tt4 in range(4):
                    for jc in range(NJT):
                        c_group(12 + tt4, jc)

    nc.compile()
    return nc


def host_prep(x, wq, wk, wv, wo):
    import ml_dtypes
    F8np = ml_dtypes.float8_e4m3
    BFnp = ml_dtypes.bfloat16

    x = np.asarray(x, dtype=np.float32)
    wq = np.asarray(wq, dtype=np.float32)
    wk = np.asarray(wk, dtype=np.float32)
    wv = np.asarray(wv, dtype=np.float32)
    wo = np.asarray(wo, dtype=np.float32)

    perm = np.concatenate([np.arange(0, D, 2), np.arange(1, D, 2)])

    inv_freq = (1.0 / THETA ** (np.arange(0, D, 2, dtype=np.float32) / D)).astype(np.float32)
    pos = np.arange(T, dtype=np.float32)
    freqs = pos[:, None] * inv_freq[None, :]
    cos_t = np.cos(freqs).astype(np.float32).T
    sin_t = np.sin(freqs).astype(np.float32).T
    dq = np.float32(16.0 / (SX * SW))
    cosT = np.concatenate([cos_t, cos_t], axis=0) * dq
    sinT = np.concatenate([-sin_t, sin_t], axis=0) * dq

    tri = (np.arange(P)[None, :] >= np.arange(P)[:, None]).astype(BFnp)

    def hilo(a):
        h = a.astype(F8np)
        l = (a - h.astype(np.float32)).astype(F8np)
        return h, l

    xs = [np.ascontiguousarray(x[b].T) * SX for b in range(B)]
    xhl = [hilo(a) for a in xs]

    in_maps = []
    for c in range(N_CORES):
        b, g = divmod(c, GROUP)
        rows = []
        for hh in range(NH):
            h = g * GROUP + hh
            rows.append(wq[h * D + perm, :])
        wq_g = np.concatenate(rows, axis=0) * SW          # [512, C]
        wk_g = wk[g * D + perm, :] * SW
        wv_g = wv[g * D:(g + 1) * D, :] * SW
        wo_g = wo[:, g * NH * D:(g + 1) * NH * D]         # [C, 512]

        wqT = np.ascontiguousarray(wq_g.T)                # [C, 512]
        wkT = np.ascontiguousarray(wk_g.T)                # [C, 128]
        wvT = np.ascontiguousarray(wv_g.T)                # [C, 128]
        rows = np.arange(C).reshape(NKP, 2, P)            # [kp, two, p]
        pk = np.concatenate([wqT[rows], wkT[rows], wvT[rows]], axis=-1)
        pk = np.ascontiguousarray(np.transpose(pk, (2, 0, 1, 3)))  # [p,kp,two,col]
        wAh, wAl = hilo(pk.reshape(P, -1))
        # wo pair-contiguous DR layout: [d, hpair, jc, h-in-pair, tc]
        woT = np.ascontiguousarray(wo_g.T) * SWO          # [512, C]
        wo_a = woT.reshape(2, 2, D, NJT, TCH)             # [i, hh, d, jc, tc]
        wo_b = np.ascontiguousarray(np.transpose(wo_a, (2, 0, 3, 1, 4)))
        woh, wol = hilo(wo_b.reshape(P, -1))

        in_maps.append({
            "xh": xhl[b][0], "xl": xhl[b][1],
            "wAh": wAh, "wAl": wAl,
            "woh": woh, "wol": wol,
            "cosT": cosT.astype(BFnp),
            "sinT": sinT.astype(BFnp),
            "tri": tri,
            "on8": np.full((P, 256), CDEN, dtype=F8np),
            "onb": np.full((P, P), CDEN, dtype=BFnp),
        })
    return in_maps


_CACHE = {}


def _get_program(key="v2"):
    if key not in _CACHE:
        _CACHE[key] = build_program()
    return _CACHE[key]


def kernel(x, mask, wq, wk, wv, wo):
    nc = _get_program()
    in_maps = host_prep(x, wq, wk, wv, wo)
    res = run_bass_kernel_spmd(nc, in_maps, list(range(N_CORES))).results
    out = np.zeros((B, T, C), dtype=np.float32)
    for c in range(N_CORES):
        out[c // GROUP] += res[c]["y"].astype(np.float32) * YDQ
    return out


# revision 13
# speedup vs baseline: 1.0556x; 1.0018x over previous
"""Causal GQA attention (B=2, T=2048, C=2048, H=16, HKV=4, D=128, RoPE)
on 8 Trainium2 NeuronCores — v2.

Sharding: core c = (batch b = c//4, kv-group g = c%4): 4 q heads + 1 kv
head per core; row-parallel output projection, host sums 4 partials.

Design notes:
  - Projections run as fp8e4 hi/lo DoubleRow matmuls (3 compensation
    terms over k-tile pairs). x and w ship pre-split/pre-scaled; dequant
    folds into the RoPE tables (q/k), the v-copy scale, and the softmax
    scale applied at exp time.
  - Attention scores also run as DoubleRow: k plain fp8 (scaled x16,
    duplicated for the stationary pair) against q in hi/lo fp8 — the two
    compensation products pack into one DoubleRow instruction, halving
    score-matmul cost. Verified end-to-end error 1.13e-2 vs the 2e-2
    gate. Remaining bf16: diagonal softmax tiles only.
  - Attention is block-causal at 128 granularity via suffix-truncated
    moving operands on the diagonal; off-diagonal s-tile pairs exp
    straight to fp8 and feed DoubleRow PV (v hi/lo) and DoubleRow
    ones-matmul denominators.
  - Output projection (bf16) is interleaved into phase B's instruction
    stream to fill the tensor engine during Act-bound stretches.
  - DMA pacing: packed projection-weight stream, wo deferred to phase
    B, trig tables per-chunk bf16 with pool-rotation gating, x
    triple-buffered.
  - RoPE: a partition-pre-swapped bf16 staging copy (Act; PSUM input
    permits the partition offset, SBUF x SBUF would not) lets the sin
    multiply run as one aligned 2-byte-mode DVE op.
"""

import os
from contextlib import ExitStack

import numpy as np

import concourse.bass as bass
import concourse.tile as tile
from concourse import bacc, mybir
from concourse.bass_utils import run_bass_kernel_spmd
from concourse.masks import make_identity

B, T, C = 2, 2048, 2048
H, HKV, D = 16, 4, 128
GROUP = H // HKV
THETA = 1000000.0
SCALE = D ** -0.5

P = 128
TCH = 512
NJT = T // TCH             # 4
NK = C // P                # 16 k-tiles
NKP = NK // 2              # 8 DoubleRow k-tile pairs
NH = GROUP                 # 4 local q heads
NST = T // P               # 16 s-tiles
N_CORES = 8

SX = 16.0                  # x fp8 scale
SW = 1024.0                # w fp8 scale (q/k/v)
SV = 2.0 ** -10            # v psum -> sbuf scale
CDEN = 0.25                # ones constant: ot = 64*O_true (fp8 range)
SWO = 1024.0               # wo fp8 scale
YDQ = 1.0 / (64.0 * SWO)   # host-side dequant of the y partials

F32 = mybir.dt.float32
BF16 = mybir.dt.bfloat16
F8 = mybir.dt.float8e4
DR = mybir.MatmulPerfMode.DoubleRow


def build_program(phases="ABC", variant=""):
    nc = bacc.Bacc("TRN2", target_bir_lowering=False, debug=False)

    xh_d = nc.dram_tensor("xh", [C, T], F8, kind="ExternalInput").ap()
    xl_d = nc.dram_tensor("xl", [C, T], F8, kind="ExternalInput").ap()
    WPK = NH * D + 2 * D       # packed row: wq 512 | wk 128 | wv 128
    wAh_d = nc.dram_tensor("wAh", [P, NKP * 2 * WPK], F8, kind="ExternalInput").ap()
    wAl_d = nc.dram_tensor("wAl", [P, NKP * 2 * WPK], F8, kind="ExternalInput").ap()
    woh_d = nc.dram_tensor("woh", [P, 2 * NJT * 2 * TCH], F8, kind="ExternalInput").ap()
    wol_d = nc.dram_tensor("wol", [P, 2 * NJT * 2 * TCH], F8, kind="ExternalInput").ap()
    cos_d = nc.dram_tensor("cosT", [P, T], BF16, kind="ExternalInput").ap()
    sin_d = nc.dram_tensor("sinT", [P, T], BF16, kind="ExternalInput").ap()
    tri_d = nc.dram_tensor("tri", [P, P], BF16, kind="ExternalInput").ap()
    on8_d = nc.dram_tensor("on8", [P, 256], F8, kind="ExternalInput").ap()
    onb_d = nc.dram_tensor("onb", [P, P], BF16, kind="ExternalInput").ap()
    y_d = nc.dram_tensor("y", [T, C], BF16, kind="ExternalOutput").ap()

    with tile.TileContext(nc) as tc, ExitStack() as ctx:
        wpool = ctx.enter_context(tc.tile_pool(name="weights", bufs=1))
        tpool = ctx.enter_context(tc.tile_pool(name="tables", bufs=1))
        trigp = ctx.enter_context(tc.tile_pool(name="trig", bufs=2))
        state = ctx.enter_context(tc.tile_pool(name="state", bufs=1))

        wAh_sb = wpool.tile([P, NKP, 2, WPK], F8, tag="wAh")
        wAl_sb = wpool.tile([P, NKP, 2, WPK], F8, tag="wAl")
        for h4 in range(4):
            nc.scalar.dma_start(
                wAh_sb[:, 2 * h4:2 * h4 + 2, :, :],
                wAh_d[:, 2 * h4 * 2 * WPK:(2 * h4 + 2) * 2 * WPK].rearrange(
                    "p (kp two o) -> p kp two o", kp=2, two=2))
        for h4 in range(4):
            nc.scalar.dma_start(
                wAl_sb[:, 2 * h4:2 * h4 + 2, :, :],
                wAl_d[:, 2 * h4 * 2 * WPK:(2 * h4 + 2) * 2 * WPK].rearrange(
                    "p (kp two o) -> p kp two o", kp=2, two=2))

        tri_sb = tpool.tile([P, P], BF16, tag="tri")
        nc.gpsimd.dma_start(tri_sb[:], tri_d[:])
        on8_sb = tpool.tile([P, 2, P], F8, tag="on8")
        nc.gpsimd.dma_start(on8_sb[:], on8_d.rearrange("p (two f) -> p two f", two=2))
        onb_sb = tpool.tile([P, P], BF16, tag="onb")
        nc.gpsimd.dma_start(onb_sb[:], onb_d[:])
        identb = tpool.tile([P, P], BF16, tag="ident")
        make_identity(nc, identb[:])
        # (variant parsed below, before use)

        # PE warmup: keep the tensor engine continuously busy from t~0 so the
        # p-state ramp completes before the first real (DMA-gated) matmuls.
        warmp = ctx.enter_context(tc.tile_pool(name="warm", bufs=1))
        warm_sb = warmp.tile([P, P], BF16, tag="w")
        nc.vector.memset(warm_sb[:], 0.0)
        nwarm = 0
        for v in variant.split("+"):
            if v.startswith("warm"):
                nwarm = int(v[4:])
        with tc.tile_pool(name="psW", bufs=1, space="PSUM") as psW:
            wp = psW.tile([P, P], F32, tag="wp")
            for i in range(nwarm):
                nc.tensor.matmul(wp[:], identb[:], warm_sb[:],
                                 start=(i == 0), stop=(i == nwarm - 1))

        qrot8 = state.tile([P, NH, 2, T], F8, tag="qrot8")
        krot8 = state.tile([P, 2, T], F8, tag="krot8")
        v_sb = state.tile([P, NST, D], BF16, tag="v")
        v8h = state.tile([P, NST // 2, 2, D], F8, tag="v8h")
        v8l = state.tile([P, NST // 2, 2, D], F8, tag="v8l")
        ot8h = state.tile([P, 2, NST, 2, D], F8, tag="ot8h")
        ot8l = state.tile([P, 2, NST, 2, D], F8, tag="ot8l")

        njt_lim = NJT
        for v in variant.split("+"):
            if v.startswith("njt"):
                njt_lim = int(v[3:])

        # ---------------- Phase A: projections + RoPE -----------------
        with ExitStack() as actx:
          if "A" in phases:
            xpool = actx.enter_context(tc.tile_pool(name="xsub", bufs=3))
            ropep = actx.enter_context(tc.tile_pool(name="rope", bufs=3))
            vtp = actx.enter_context(tc.tile_pool(name="vt", bufs=2))
            psA = actx.enter_context(tc.tile_pool(name="psA", bufs=1, space="PSUM"))
            psT = actx.enter_context(tc.tile_pool(name="psT", bufs=2, space="PSUM"))

            def rope(acc_ps, cos_t, sin_t):
                # partition-pre-swapped bf16 stage (Act, PSUM input permits the
                # offset) so both DVE multiplies run aligned in 2-byte mode
                absw = ropep.tile([P, TCH], BF16, tag="ab")
                nc.scalar.copy(absw[0:64, :], acc_ps[64:128, :])
                nc.scalar.copy(absw[64:128, :], acc_ps[0:64, :])
                m1 = ropep.tile([P, TCH], BF16, tag="m1")
                m2 = ropep.tile([P, TCH], BF16, tag="m2")
                nc.vector.tensor_tensor(
                    m1[:], acc_ps[:], cos_t[:], mybir.AluOpType.mult)
                nc.vector.tensor_tensor(
                    m2[:], absw[:], sin_t[:], mybir.AluOpType.mult)
                out = ropep.tile([P, TCH], BF16, tag="ro")
                nc.vector.tensor_tensor(
                    out[:], m1[:], m2[:], mybir.AluOpType.add)
                return out

            def w_slice(term, o, m):
                sb = wAh_sb if term != 2 else wAl_sb
                if o < NH:
                    return sb[:, m, :, o * D:(o + 1) * D]
                if o == NH:
                    return sb[:, m, :, NH * D:NH * D + D]
                return sb[:, m, :, NH * D + D:NH * D + 2 * D]

            def finish(o, acc, jt, cos_t, sin_t):
                """Returns a deferred-emission thunk (or None)."""
                if "noropeA" in variant:
                    return None
                ch_ = slice(jt * TCH, (jt + 1) * TCH)
                if o < NH:
                    qt = rope(acc, cos_t, sin_t)
                    qh_ap = qrot8[:, o, 0, ch_]
                    nc.vector.tensor_copy(qh_ap, qt[:])
                    nc.vector.tensor_tensor(
                        qrot8[:, o, 1, ch_], qt[:], qh_ap,
                        mybir.AluOpType.subtract)
                    return None
                if o == NH:
                    kt = rope(acc, cos_t, sin_t)
                    nc.scalar.copy(krot8[:, 0, ch_], kt[:])
                    nc.scalar.copy(krot8[:, 1, ch_], kt[:])
                    return None
                vt = vtp.tile([P, TCH], BF16, tag="vt")
                nc.scalar.activation(
                    vt[:], acc[:], mybir.ActivationFunctionType.Copy, scale=SV)

                def transposes():
                    for i in range(TCH // P):
                        s_idx = jt * (TCH // P) + i
                        pst = psT.tile([P, P], BF16, tag="pst")
                        nc.tensor.transpose(pst[:], vt[:, i * P:(i + 1) * P],
                                            identb[:])
                        nc.scalar.copy(v_sb[:, s_idx, :], pst[:])
                        nc.scalar.copy(v8h[:, s_idx // 2, s_idx % 2, :], pst[:])
                        nc.vector.tensor_tensor(
                            v8l[:, s_idx // 2, s_idx % 2, :], pst[:],
                            v8h[:, s_idx // 2, s_idx % 2, :],
                            mybir.AluOpType.subtract)
                return transposes

            nacc = 0
            deferred = None
            for jt in range(njt_lim):
                cos_t = trigp.tile([P, TCH], BF16, tag="cos")
                sin_t = trigp.tile([P, TCH], BF16, tag="sin")
                ch = slice(jt * TCH, (jt + 1) * TCH)
                nc.gpsimd.dma_start(cos_t[:], cos_d[:, ch])
                nc.gpsimd.dma_start(sin_t[:], sin_d[:, ch])

                xhs, xls = [], []
                for m in range(NKP):
                    xt = xpool.tile([P, 2, TCH], F8, tag=f"xh{m}")
                    nc.sync.dma_start(
                        xt[:],
                        xh_d[2 * m * P:(2 * m + 2) * P, ch].rearrange(
                            "(two p) t -> p two t", p=P))
                    xhs.append(xt)
                for m in range(NKP):
                    xt = xpool.tile([P, 2, TCH], F8, tag=f"xl{m}")
                    nc.sync.dma_start(
                        xt[:],
                        xl_d[2 * m * P:(2 * m + 2) * P, ch].rearrange(
                            "(two p) t -> p two t", p=P))
                    xls.append(xt)

                if jt == 0:
                    accs = [psA.tile([P, TCH], F32, tag=f"acc{o}",
                                     name=f"acc{o}")
                            for o in range(6)]
                    for ti, (xs, term) in enumerate(((xhs, 0), (xls, 1),
                                                     (xhs, 2))):
                        for m in range(NKP):
                            for o in (4, 5, 0, 1, 2, 3):
                                nc.tensor.matmul(
                                    accs[o][:], w_slice(term, o, m),
                                    xs[m][:],
                                    start=(ti == 0 and m == 0),
                                    stop=(ti == 2 and m == NKP - 1),
                                    perf_mode=DR)
                    for o in (5, 4, 0, 1, 2, 3):
                        d = finish(o, accs[o], jt, cos_t, sin_t)
                        if d is not None:
                            deferred = d
                    nacc = 5
                else:
                    for o in (4, 5, 0, 1, 2, 3):
                        acc = psA.tile([P, TCH], F32, tag=f"acc{nacc % 6}",
                                       name=f"accr{nacc % 6}")
                        nacc += 1
                        first = True
                        for xs, term in (((xhs, 0), (xls, 1), (xhs, 2))):
                            for m in range(NKP):
                                nc.tensor.matmul(
                                    acc[:], w_slice(term, o, m), xs[m][:],
                                    start=first,
                                    stop=(term == 2 and m == NKP - 1),
                                    perf_mode=DR)
                                first = False
                        if deferred is not None:
                            deferred()
                            deferred = None
                        d = finish(o, acc, jt, cos_t, sin_t)
                        if d is not None:
                            deferred = d
                if jt == njt_lim - 1 and deferred is not None:
                    deferred()
                    deferred = None

        # ---------------- Phase B + C interleaved ----------------------
        with ExitStack() as bctx:
          if "B" in phases:
            e8p = bctx.enter_context(tc.tile_pool(name="e8", bufs=6))
            edp = bctx.enter_context(tc.tile_pool(name="ed", bufs=3))
            rcp = bctx.enter_context(tc.tile_pool(name="rc", bufs=3))
            wopool = bctx.enter_context(tc.tile_pool(name="wo", bufs=1))
            ypool = bctx.enter_context(tc.tile_pool(name="ysb", bufs=6))
            psS = bctx.enter_context(tc.tile_pool(name="psS", bufs=2, space="PSUM"))
            psO = bctx.enter_context(tc.tile_pool(name="psO", bufs=1, space="PSUM"))
            psD = bctx.enter_context(tc.tile_pool(name="psD", bufs=1, space="PSUM"))
            psC = bctx.enter_context(tc.tile_pool(name="psC", bufs=2, space="PSUM"))

            woh_sb = wopool.tile([P, 2, NJT, 2, TCH], F8, tag="woh")
            wol_sb = wopool.tile([P, 2, NJT, 2, TCH], F8, tag="wol")
            nc.sync.dma_start(
                woh_sb[:], woh_d.rearrange("p (i jc two t) -> p i jc two t",
                                           i=2, jc=NJT, two=2))
            nc.sync.dma_start(
                wol_sb[:], wol_d.rearrange("p (i jc two t) -> p i jc two t",
                                           i=2, jc=NJT, two=2))

            def c_group(tt, jc):
                yp = psC.tile([P, TCH], F32, tag="y")
                first = True
                for i in range(2):
                    for lhs, rhs in ((ot8h, woh_sb), (ot8l, woh_sb),
                                     (ot8h, wol_sb)):
                        nc.tensor.matmul(
                            yp[:], lhs[:, i, tt, :, :], rhs[:, i, jc, :, :],
                            start=first, stop=(i == 1 and rhs is wol_sb),
                            perf_mode=DR, skip_group_check=True)
                        first = False
                ys = ypool.tile([P, TCH], BF16, tag="ys")
                nc.vector.tensor_copy(ys[:], yp[:])
                nc.sync.dma_start(
                    y_d[tt * P:(tt + 1) * P, jc * TCH:(jc + 1) * TCH],
                    ys[:])

            do_c = "C" in phases

            nbjt = NJT
            for v in variant.split("+"):
                if v.startswith("nbjt"):
                    nbjt = int(v[4:])
            cpend = []

            def emit_c(n=1):
                for _ in range(n):
                    if cpend:
                        c_group(*cpend.pop(0))

            for jt in range(nbjt):
                ch = slice(jt * TCH, (jt + 1) * TCH)
                if do_c and jt >= 1:
                    cpend = [(4 * (jt - 1) + tt4, jc)
                             for tt4 in range(4) for jc in range(NJT)]
                for h in range(NH):
                    qch = qrot8[:, h, :, ch]
                    ot_ps = psO.tile([P, TCH], F32, tag="ot")
                    dn_ps = psD.tile([P, TCH], F32, tag="dn")
                    npair = 2 * jt

                    def sc_pair(m):
                        sps = psS.tile([P, 2 * TCH], F32, tag="su")
                        for i in range(2):
                            js = 2 * m + i
                            nc.tensor.matmul(
                                sps[:, i * TCH:(i + 1) * TCH],
                                krot8[:, :, js * P:(js + 1) * P], qch,
                                start=True, stop=True, perf_mode=DR,
                                skip_group_check=True)
                        e8 = e8p.tile([P, 2, TCH], F8, tag="e8")
                        nc.scalar.activation(
                            e8[:], sps[:], mybir.ActivationFunctionType.Exp,
                            scale=SCALE / 256.0)
                        return e8

                    def pv_pair(m, e8, start):
                        nc.tensor.matmul(
                            ot_ps[:], v8h[:, m, :, :], e8[:],
                            start=start, stop=False, perf_mode=DR,
                            skip_group_check=True)
                        nc.tensor.matmul(
                            ot_ps[:], v8l[:, m, :, :], e8[:],
                            start=False, stop=False, perf_mode=DR,
                            skip_group_check=True)
                        nc.tensor.matmul(
                            dn_ps[:], on8_sb[:], e8[:],
                            start=start, stop=False, perf_mode=DR,
                            skip_group_check=True)

                    pend = []
                    for m in range(npair):
                        e8 = sc_pair(m)
                        if len(pend) >= 4:
                            mm, ee = pend.pop(0)
                            pv_pair(mm, ee, mm == 0)
                        pend.append((m, e8))

                    dsup = []
                    es8d = jt >= 1   # chunk-0 diagonal stays bf16 (peaked rows)
                    for half in range(2):
                        sps = psS.tile([P, 2 * TCH], F32, tag="su")
                        if es8d:
                            ed = edp.tile([P, 2 * TCH], F8, tag="ed8",
                                          name="ed8")
                        else:
                            ed = edp.tile([P, 2 * TCH], BF16, tag="ed",
                                          name="ed")
                        widths = []
                        off = 0
                        for rr in range(2):
                            r = 2 * half + rr
                            w = TCH - r * P
                            js = 4 * jt + r
                            nc.tensor.matmul(
                                sps[:, off:off + w],
                                krot8[:, :, js * P:(js + 1) * P],
                                qch[:, :, r * P:],
                                start=True, stop=True, perf_mode=DR,
                                skip_group_check=True)
                            widths.append((r, off, w))
                            off += w
                        nc.scalar.activation(
                            ed[:, 0:off], sps[:, 0:off],
                            mybir.ActivationFunctionType.Exp, scale=SCALE / 256.0)
                        for r, off_, w in widths:
                            nc.vector.tensor_tensor(
                                ed[:, off_:off_ + P], ed[:, off_:off_ + P],
                                tri_sb[:], mybir.AluOpType.mult)
                        dsup.append((ed, widths))
                        while pend:
                            mm, ee = pend.pop(0)
                            pv_pair(mm, ee, mm == 0)

                    if es8d:
                        # fp8 diagonal: DoubleRow pairs over adjacent suffix
                        # regions (the super layout makes cols [128:896] of
                        # super0 a contiguous [2,384] pair, [128:384] of
                        # super1 a [2,128] pair) + plain-fp8 singles for the
                        # leading 128-col leftovers
                        ed0, ed1 = dsup[0][0], dsup[1][0]
                        pa = ed0[:, P:P + 768].rearrange(
                            "p (two w) -> p two w", two=2)
                        pb = ed1[:, P:P + 256].rearrange(
                            "p (two w) -> p two w", two=2)
                        for vv in (v8h, v8l):
                            nc.tensor.matmul(
                                ot_ps[:, P:], vv[:, 2 * jt, :, :], pa,
                                start=False, stop=False, perf_mode=DR,
                                skip_group_check=True)
                            nc.tensor.matmul(
                                ot_ps[:, 3 * P:], vv[:, 2 * jt + 1, :, :], pb,
                                start=False, stop=False, perf_mode=DR,
                                skip_group_check=True)
                            nc.tensor.matmul(
                                ot_ps[:, 0:P], vv[:, 2 * jt, 0, :],
                                ed0[:, 0:P], start=False, stop=False,
                                skip_group_check=True)
                            nc.tensor.matmul(
                                ot_ps[:, 2 * P:3 * P], vv[:, 2 * jt + 1, 0, :],
                                ed1[:, 0:P], start=False, stop=(vv is v8l),
                                skip_group_check=True)
                        nc.tensor.matmul(
                            dn_ps[:, P:], on8_sb[:], pa,
                            start=False, stop=False, perf_mode=DR,
                            skip_group_check=True)
                        nc.tensor.matmul(
                            dn_ps[:, 3 * P:], on8_sb[:], pb,
                            start=False, stop=False, perf_mode=DR,
                            skip_group_check=True)
                        nc.tensor.matmul(
                            dn_ps[:, 0:P], on8_sb[:, 0, :], ed0[:, 0:P],
                            start=False, stop=False, skip_group_check=True)
                        nc.tensor.matmul(
                            dn_ps[:, 2 * P:3 * P], on8_sb[:, 0, :],
                            ed1[:, 0:P], start=False, stop=True,
                            skip_group_check=True)
                    else:
                        for ed, widths in dsup:
                            for r, off, w in widths:
                                js = 4 * jt + r
                                nc.tensor.matmul(
                                    ot_ps[:, r * P:], v_sb[:, js, :],
                                    ed[:, off:off + w],
                                    start=(jt == 0 and r == 0), stop=(r == 3),
                                    skip_group_check=True)
                                nc.tensor.matmul(
                                    dn_ps[:, r * P:], onb_sb[:],
                                    ed[:, off:off + w],
                                    start=(jt == 0 and r == 0), stop=(r == 3),
                                    skip_group_check=True)

                    if not (jt == NJT - 1 and h == NH - 1):
                        emit_c(4)
                    rb = rcp.tile([P, TCH], F32, tag="rb")
                    nc.vector.reciprocal(rb[:], dn_ps[:])
                    tmp = rcp.tile([P, 4, P], F32, tag="tmp")
                    nc.vector.tensor_tensor(
                        tmp[:].rearrange("p a b -> p (a b)"), ot_ps[:], rb[:],
                        mybir.AluOpType.mult)
                    oh = ot8h[:, h // 2, 4 * jt:4 * jt + 4, h % 2, :]
                    ol = ot8l[:, h // 2, 4 * jt:4 * jt + 4, h % 2, :]
                    nc.vector.tensor_copy(oh, tmp[:])
                    nc.vector.tensor_tensor(
                        ol, tmp[:], oh, mybir.AluOpType.subtract)
                    if jt == NJT - 1 and h == NH - 1:
                        emit_c(4)


            if do_c:
                for tt4 in range(4):
                    for jc in range(NJT):
                        c_group(12 + tt4, jc)

    nc.compile()
    return nc


def host_prep(x, wq, wk, wv, wo):
    import ml_dtypes
    F8np = ml_dtypes.float8_e4m3
    BFnp = ml_dtypes.bfloat16

    x = np.asarray(x, dtype=np.float32)
    wq = np.asarray(wq, dtype=np.float32)
    wk = np.asarray(wk, dtype=np.float32)
    wv = np.asarray(wv, dtype=np.float32)
    wo = np.asarray(wo, dtype=np.float32)

    perm = np.concatenate([np.arange(0, D, 2), np.arange(1, D, 2)])

    inv_freq = (1.0 / THETA ** (np.arange(0, D, 2, dtype=np.float32) / D)).astype(np.float32)
    pos = np.arange(T, dtype=np.float32)
    freqs = pos[:, None] * inv_freq[None, :]
    cos_t = np.cos(freqs).astype(np.float32).T
    sin_t = np.sin(freqs).astype(np.float32).T
    dq = np.float32(16.0 / (SX * SW))
    cosT = np.concatenate([cos_t, cos_t], axis=0) * dq
    sinT = np.concatenate([-sin_t, sin_t], axis=0) * dq

    tri = (np.arange(P)[None, :] >= np.arange(P)[:, None]).astype(BFnp)

    def hilo(a):
        h = a.astype(F8np)
        l = (a - h.astype(np.float32)).astype(F8np)
        return h, l

    xs = [np.ascontiguousarray(x[b].T) * SX for b in range(B)]
    xhl = [hilo(a) for a in xs]

    in_maps = []
    for c in range(N_CORES):
        b, g = divmod(c, GROUP)
        rows = []
        for hh in range(NH):
            h = g * GROUP + hh
            rows.append(wq[h * D + perm, :])
        wq_g = np.concatenate(rows, axis=0) * SW          # [512, C]
        wk_g = wk[g * D + perm, :] * SW
        wv_g = wv[g * D:(g + 1) * D, :] * SW
        wo_g = wo[:, g * NH * D:(g + 1) * NH * D]         # [C, 512]

        wqT = np.ascontiguousarray(wq_g.T)                # [C, 512]
        wkT = np.ascontiguousarray(wk_g.T)                # [C, 128]
        wvT = np.ascontiguousarray(wv_g.T)                # [C, 128]
        rows = np.arange(C).reshape(NKP, 2, P)            # [kp, two, p]
        pk = np.concatenate([wqT[rows], wkT[rows], wvT[rows]], axis=-1)
        pk = np.ascontiguousarray(np.transpose(pk, (2, 0, 1, 3)))  # [p,kp,two,col]
        wAh, wAl = hilo(pk.reshape(P, -1))
        # wo pair-contiguous DR layout: [d, hpair, jc, h-in-pair, tc]
        woT = np.ascontiguousarray(wo_g.T) * SWO          # [512, C]
        wo_a = woT.reshape(2, 2, D, NJT, TCH)             # [i, hh, d, jc, tc]
        wo_b = np.ascontiguousarray(np.transpose(wo_a, (2, 0, 3, 1, 4)))
        woh, wol = hilo(wo_b.reshape(P, -1))

        in_maps.append({
            "xh": xhl[b][0], "xl": xhl[b][1],
            "wAh": wAh, "wAl": wAl,
            "woh": woh, "wol": wol,
            "cosT": cosT.astype(BFnp),
            "sinT": sinT.astype(BFnp),
            "tri": tri,
            "on8": np.full((P, 256), CDEN, dtype=F8np),
            "onb": np.full((P, P), CDEN, dtype=BFnp),
        })
    return in_maps


_CACHE = {}


def _get_program(key="v2"):
    if key not in _CACHE:
        _CACHE[key] = build_program()
    return _CACHE[key]


def kernel(x, mask, wq, wk, wv, wo):
    nc = _get_program()
    in_maps = host_prep(x, wq, wk, wv, wo)
    res = run_bass_kernel_spmd(nc, in_maps, list(range(N_CORES))).results
    out = np.zeros((B, T, C), dtype=np.float32)
    for c in range(N_CORES):
        out[c // GROUP] += res[c]["y"].astype(np.float32) * YDQ
    return out
